# revision 17
# baseline (speedup 1.0000x reference)
"""Distributed Trainium2 Bass kernel for nn_Attention_72791105732731.

Reference computation (S=16384, H=4096):
    score_ = hidden @ W1.T            # [S,H]
    h_t    = hidden[-1]
    score  = score_ @ h_t             # [S]
    aw     = softmax(score)
    ctx    = hidden.T @ aw            # [H]
    av     = tanh(W2 @ concat(ctx, h_t))
    return (av, aw)

Key algebraic identity: score = hidden @ (W1.T @ h_t) — reassociation turns the
550-GFLOP fc1 matmul into two matvecs, making the problem memory-bound.

Distribution over 8 cores:
  - hidden sharded over S (2048 rows/core), host-pre-transposed to [H, 2048]
    so the score contraction (over H) sits on the partition axis for TensorE.
  - W1 sharded over rows (512/core): partial v = W1_shard.T @ h_t_shard,
    AllGather + local sum (cheaper than AllReduce at this size).
  - softmax via block-local (max, sumexp) stats + AllGather of per-core stats.
  - context partials: DVE multiplies (bf16 2x mode, fused 8 tiles/op), free-dim
    sums split between the Scalar engine (activation accum_out) and GPSIMD so
    no single engine paces the pass; AllGather + local sum for the context.
  - W2 sharded over output rows (512/core), host-pre-transposed; the h_t half
    of fc2 is accumulated into PSUM during the main pass, only the ctx half
    remains after the context AllGather.

Compute dtype bf16 (validated offline: aw absmax err ~7e-5, av ~9e-3 vs fp32
reference); all accumulations fp32 (PSUM / ACT accumulator / stats math).
"""

from contextlib import ExitStack

import ml_dtypes
import numpy as np

import concourse.bass as bass
import concourse.tile as tile
from concourse import bacc, mybir
from concourse.bass_utils import run_bass_kernel_spmd

F32 = mybir.dt.float32
BF16 = mybir.dt.bfloat16
AF = mybir.ActivationFunctionType
ALU = mybir.AluOpType

N_CORES = 8
S = 16384
H = 4096


def build_graph(n_cores=N_CORES, s_shard=S // N_CORES, h=H, blk=512,
                m_shard=H // N_CORES, act_red=20, dve_red=12, fuse=8):
    """Build the SPMD single-core Bass graph (identical on every core)."""
    nb = s_shard // blk          # score/softmax blocks per core
    ht_tiles = h // 128          # h-tiles (partition tiles)
    pm_cols = h // 128           # columns of the partition-major h_t layout
    jt = (h // n_cores) // 128   # W1 row tiles per core
    k2 = 2 * h // 128            # fc2 contraction tiles
    sub = 4                      # hidden sub-DMAs per block
    assert ht_tiles % sub == 0 and ht_tiles % fuse == 0

    nc = bacc.Bacc("TRN2", target_bir_lowering=False, debug=False,
                   num_devices=n_cores)

    # ---- I/O ----
    hid_t = nc.dram_tensor("hid_t", [h, s_shard], BF16, kind="ExternalInput")
    w1s = nc.dram_tensor("w1s", [h // n_cores, h], BF16, kind="ExternalInput")
    w2t = nc.dram_tensor("w2t", [2 * h, m_shard], BF16, kind="ExternalInput")
    ht_pm = nc.dram_tensor("ht_pm", [128, pm_cols], BF16, kind="ExternalInput")
    ht_loc = nc.dram_tensor("ht_loc", [128, jt], BF16, kind="ExternalInput")
    out_w = nc.dram_tensor("out_w", [s_shard], F32, kind="ExternalOutput")
    out_av = nc.dram_tensor("out_av", [m_shard], F32, kind="ExternalOutput")

    groups = [list(range(n_cores))]

    with tile.TileContext(nc) as tc, ExitStack() as ctx:
        dram = ctx.enter_context(tc.tile_pool(name="dram", bufs=1, space="DRAM"))
        psum = ctx.enter_context(tc.tile_pool(name="psum", bufs=1, space="PSUM"))
        psum2 = ctx.enter_context(tc.tile_pool(name="psum2", bufs=2, space="PSUM"))
        sb = ctx.enter_context(tc.tile_pool(name="sb", bufs=1))
        sb2 = ctx.enter_context(tc.tile_pool(name="sb2", bufs=2))
        hidp = ctx.enter_context(tc.tile_pool(name="hidp", bufs=2))
        w1p = ctx.enter_context(tc.tile_pool(name="w1p", bufs=1))

        ones_bf = sb.tile([1, 128], BF16, name="ones_bf")
        nc.vector.memset(ones_bf[:], 1.0)
        ones_f32 = sb.tile([1, 128], F32, name="ones_f32")
        nc.vector.memset(ones_f32[:], 1.0)

        ht_loc_sb = sb.tile([128, jt], BF16, name="ht_loc_sb")
        nc.sync.dma_start(ht_loc_sb[:], ht_loc.ap())
        ht_pm_sb = sb.tile([128, pm_cols], BF16, name="ht_pm_sb")
        nc.sync.dma_start(ht_pm_sb[:], ht_pm.ap())

        # ---- v = W1.T @ h_t (partial over this core's W1 rows) ----
        # One accumulation group per PSUM bank at a time: each v column gets
        # its own psum tile (2-buf rotation) and is drained by ACT before the
        # bank is re-used.
        w1_sbs = []
        for j in range(jt):
            w1_sb = w1p.tile([128, h], BF16, name=f"w1_sb{j}", tag=f"w1{j}")
            nc.sync.dma_start(w1_sb[:], w1s.ap()[j * 128:(j + 1) * 128, :])
            w1_sbs.append(w1_sb)
        v_sb = sb.tile([128, pm_cols], F32, name="v_sb")
        for i in range(ht_tiles):
            v_ps = psum2.tile([128, 1], F32, name="v_ps", tag="vps")
            for j in range(jt):
                nc.tensor.matmul(
                    v_ps[:],
                    lhsT=w1_sbs[j][:, i * 128:(i + 1) * 128],
                    rhs=ht_loc_sb[:, j:j + 1],
                    start=(j == 0), stop=(j == jt - 1),
                )
            nc.scalar.copy(v_sb[:, i:i + 1], v_ps[:])

        # v partial -> AllGather -> local sum over ranks
        v_bounce = dram.tile([128 * pm_cols], F32, name="v_bounce")
        v_gath = dram.tile([n_cores * 128 * pm_cols], F32, name="v_gath")
        nc.sync.dma_start(v_bounce[:].rearrange("(p t) -> p t", p=128), v_sb[:])
        nc.gpsimd.collective_compute(
            "AllGather", ALU.bypass, replica_groups=groups,
            ins=[v_bounce.opt()], outs=[v_gath.opt()],
        )
        v_all = sb.tile([128, pm_cols * n_cores], F32, name="v_all")
        nc.sync.dma_start(
            v_all[:].rearrange("p (t r) -> p t r", r=n_cores),
            v_gath[:].rearrange("(r p t) -> p t r", p=128, t=pm_cols))
        v_rd = sb.tile([128, pm_cols], F32, name="v_rd")
        nc.vector.reduce_sum(
            out=v_rd[:],
            in_=v_all[:].rearrange("p (t r) -> p t r", r=n_cores),
            axis=mybir.AxisListType.X)
        v_pm = sb.tile([128, pm_cols], BF16, name="v_pm")
        nc.scalar.copy(v_pm[:], v_rd[:])

        # ---- main pass over s-blocks: score -> exp -> context partials ----
        e_rows = sb.tile([1, s_shard], F32, name="e_rows")
        mb_row = sb.tile([1, nb], F32, name="mb_row")
        negmb_row = sb.tile([1, nb], F32, name="negmb_row")
        zb_row = sb.tile([1, nb], F32, name="zb_row")
        ctx_store = sb.tile([128, ht_tiles * nb], F32, name="ctx_store")

        for b in range(nb):
            hid_sb = hidp.tile([128, ht_tiles * blk], BF16, name="hid_sb",
                               tag="hid")
            for g in range(sub):
                tpg = ht_tiles // sub  # h-tiles per sub-DMA
                nc.sync.dma_start(
                    hid_sb[:, g * tpg * blk:(g + 1) * tpg * blk]
                    .rearrange("p (t s) -> p t s", t=tpg),
                    hid_t.ap()[g * tpg * 128:(g + 1) * tpg * 128,
                               b * blk:(b + 1) * blk]
                    .rearrange("(t p) s -> p t s", p=128),
                )

            score_ps = psum2.tile([1, blk], F32, name="score_ps", tag="score")
            for t in range(ht_tiles):
                nc.tensor.matmul(
                    score_ps[:],
                    lhsT=v_pm[:, t:t + 1],
                    rhs=hid_sb[:, t * blk:(t + 1) * blk],
                    start=(t == 0), stop=(t == ht_tiles - 1),
                )

            nc.vector.reduce_max(out=mb_row[:, b:b + 1], in_=score_ps[:],
                                 axis=mybir.AxisListType.X)
            nc.scalar.mul(negmb_row[:, b:b + 1], mb_row[:, b:b + 1], -1.0)
            # e = exp(score - m_b); Z_b accumulated by the activation engine
            nc.scalar.activation(
                e_rows[:, b * blk:(b + 1) * blk], score_ps[:], AF.Exp,
                bias=negmb_row[:, b:b + 1], scale=1.0,
                accum_out=zb_row[:, b:b + 1],
            )
            # broadcast e across partitions via PE (bf16), cast to bf16
            e_bf = sb2.tile([1, blk], BF16, name="e_bf", tag="ebf")
            nc.scalar.copy(e_bf[:], e_rows[:, b * blk:(b + 1) * blk])
            e_bc_ps = psum2.tile([128, blk], F32, name="e_bc_ps", tag="ebc")
            nc.tensor.matmul(e_bc_ps[:], lhsT=ones_bf[:], rhs=e_bf[:],
                             start=True, stop=True)
            e_bc = sb2.tile([128, blk], BF16, name="e_bc", tag="ebc_sb")
            nc.scalar.copy(e_bc[:], e_bc_ps[:])

            # context partials: ctx_store[:, b*HT + t] = sum_s hid*e
            # DVE does fused multiplies; the per-tile free-dim sums are split
            # between ACT (activation accum_out) and GPSIMD.
            e_rep = (e_bc[:].rearrange("p (o s) -> p o s", o=1)
                     .broadcast_to((128, fuse, blk)))
            for f in range(ht_tiles // fuse):
                tt_out = sb2.tile([128, fuse * blk], BF16, name="tt_out",
                                  tag="tt", bufs=2)
                nc.vector.tensor_mul(
                    tt_out[:].rearrange("p (t s) -> p t s", t=fuse),
                    hid_sb[:, f * fuse * blk:(f + 1) * fuse * blk]
                    .rearrange("p (t s) -> p t s", t=fuse),
                    e_rep)
                for ti in range(fuse):
                    t = f * fuse + ti
                    col = ctx_store[:, b * ht_tiles + t:b * ht_tiles + t + 1]
                    src = tt_out[:, ti * blk:(ti + 1) * blk]
                    if t < act_red:
                        junk_ps = psum2.tile([128, blk], F32, name="junk_ps",
                                             tag="junk", bufs=1)
                        nc.scalar.activation(junk_ps[:], src, AF.Copy,
                                             accum_out=col)
                    else:
                        nc.vector.reduce_sum(out=col, in_=src,
                                             axis=mybir.AxisListType.X)

        # ---- fc2 h_t half: accumulate into PSUM while stats/collectives run
        w2_sb = sb.tile([128, k2 * m_shard], BF16, name="w2_sb")
        w2_sub = 8
        tpg2 = k2 // w2_sub
        for g in range(w2_sub):
            nc.sync.dma_start(
                w2_sb[:, g * tpg2 * m_shard:(g + 1) * tpg2 * m_shard]
                .rearrange("p (t m) -> p t m", t=tpg2),
                w2t.ap()[g * tpg2 * 128:(g + 1) * tpg2 * 128, :]
                .rearrange("(t p) m -> p t m", p=128),
            )
        fc2_ps = psum.tile([1, m_shard], F32, name="fc2_ps")
        for k in range(pm_cols, k2):
            nc.tensor.matmul(
                fc2_ps[:],
                lhsT=ht_pm_sb[:, k - pm_cols:k - pm_cols + 1],
                rhs=w2_sb[:, k * m_shard:(k + 1) * m_shard],
                start=(k == pm_cols), stop=False,
            )

        # ---- local stats -> AllGather -> global softmax factors ----
        neg_m_core = sb.tile([1, 1], F32, name="neg_m_core")
        nc.vector.tensor_reduce(out=neg_m_core[:], in_=mb_row[:],
                                axis=mybir.AxisListType.X, op=ALU.max,
                                negate=True)
        alpha = sb.tile([1, nb], F32, name="alpha")
        nc.scalar.activation(alpha[:], negmb_row[:], AF.Exp,
                             bias=neg_m_core[:], scale=-1.0)
        scr_nb = sb.tile([1, nb], F32, name="scr_nb")
        z_core = sb.tile([1, 1], F32, name="z_core")
        nc.vector.tensor_mul(scr_nb[:], alpha[:], zb_row[:])
        nc.vector.reduce_sum(out=z_core[:], in_=scr_nb[:],
                             axis=mybir.AxisListType.X)
        stats_sb = sb.tile([1, 2], F32, name="stats_sb")
        nc.scalar.mul(stats_sb[:, 0:1], neg_m_core[:], -1.0)
        nc.scalar.copy(stats_sb[:, 1:2], z_core[:])

        stats_bounce = dram.tile([2], F32, name="stats_bounce")
        stats_all = dram.tile([2 * n_cores], F32, name="stats_all")
        nc.sync.dma_start(stats_bounce[:].rearrange("(p f) -> p f", p=1),
                          stats_sb[:])
        nc.gpsimd.collective_compute(
            "AllGather", ALU.bypass, replica_groups=groups,
            ins=[stats_bounce.opt()], outs=[stats_all.opt()],
        )
        m_all = sb.tile([1, n_cores], F32, name="m_all")
        z_all = sb.tile([1, n_cores], F32, name="z_all")
        strided = stats_all[:].rearrange("(r two) -> two r", two=2)
        nc.sync.dma_start(m_all[:], strided[0:1, :])
        nc.sync.dma_start(z_all[:], strided[1:2, :])

        neg_m_g = sb.tile([1, 1], F32, name="neg_m_g")
        nc.vector.tensor_reduce(out=neg_m_g[:], in_=m_all[:],
                                axis=mybir.AxisListType.X, op=ALU.max,
                                negate=True)
        beta = sb.tile([1, n_cores], F32, name="beta")
        nc.scalar.activation(beta[:], m_all[:], AF.Exp, bias=neg_m_g[:],
                             scale=1.0)
        scr_nc = sb.tile([1, n_cores], F32, name="scr_nc")
        z_g = sb.tile([1, 1], F32, name="z_g")
        nc.vector.tensor_mul(scr_nc[:], beta[:], z_all[:])
        nc.vector.reduce_sum(out=z_g[:], in_=scr_nc[:],
                             axis=mybir.AxisListType.X)
        inv_zg = sb.tile([1, 1], F32, name="inv_zg")
        nc.vector.reciprocal(inv_zg[:], z_g[:])
        # gamma_b = exp(m_b - m_g) / Z_g
        gamma = sb.tile([1, nb], F32, name="gamma")
        nc.scalar.activation(gamma[:], negmb_row[:], AF.Exp, bias=neg_m_g[:],
                             scale=-1.0)
        gamma2 = sb.tile([1, nb], F32, name="gamma2")
        nc.vector.tensor_scalar_mul(gamma2[:], gamma[:], inv_zg[:])

        # ---- attention weights output ----
        w_row = sb.tile([1, s_shard], F32, name="w_row")
        for b in range(nb):
            nc.vector.tensor_scalar_mul(w_row[:, b * blk:(b + 1) * blk],
                                        e_rows[:, b * blk:(b + 1) * blk],
                                        gamma2[:, b:b + 1])
        nc.sync.dma_start(out_w.ap().rearrange("(p f) -> p f", p=1), w_row[:])

        # ---- combine context partials, AllGather + local sum ----
        gam_ps = psum2.tile([128, nb], F32, name="gam_ps", tag="vps")
        nc.tensor.matmul(gam_ps[:], lhsT=ones_f32[:], rhs=gamma2[:],
                         start=True, stop=True)
        gam_sb = sb.tile([128, nb], F32, name="gam_sb")
        nc.scalar.copy(gam_sb[:], gam_ps[:])

        ctx_acc = sb.tile([128, ht_tiles], F32, name="ctx_acc")
        ctx_tmp = sb.tile([128, ht_tiles], F32, name="ctx_tmp")
        nc.vector.tensor_scalar_mul(
            ctx_acc[:], ctx_store[:, 0:ht_tiles], gam_sb[:, 0:1])
        for b in range(1, nb):
            nc.vector.tensor_scalar_mul(
                ctx_tmp[:], ctx_store[:, b * ht_tiles:(b + 1) * ht_tiles],
                gam_sb[:, b:b + 1])
            nc.vector.tensor_add(ctx_acc[:], ctx_acc[:], ctx_tmp[:])

        ctx_bounce = dram.tile([h], F32, name="ctx_bounce")
        ctx_gath = dram.tile([n_cores * h], F32, name="ctx_gath")
        nc.sync.dma_start(ctx_bounce[:].rearrange("(p t) -> p t", p=128),
                          ctx_acc[:])
        nc.gpsimd.collective_compute(
            "AllGather", ALU.bypass, replica_groups=groups,
            ins=[ctx_bounce.opt()], outs=[ctx_gath.opt()],
        )
        ctx_all = sb.tile([128, ht_tiles * n_cores], F32, name="ctx_all")
        nc.sync.dma_start(
            ctx_all[:].rearrange("p (t r) -> p t r", r=n_cores),
            ctx_gath[:].rearrange("(r p t) -> p t r", p=128, t=ht_tiles))
        ctx_rd = sb.tile([128, ht_tiles], F32, name="ctx_rd")
        nc.vector.reduce_sum(
            out=ctx_rd[:],
            in_=ctx_all[:].rearrange("p (t r) -> p t r", r=n_cores),
            axis=mybir.AxisListType.X)

        # ---- fc2 ctx half + tanh ----
        ctx_bf = sb.tile([128, ht_tiles], BF16, name="ctx_bf")
        nc.scalar.copy(ctx_bf[:], ctx_rd[:])
        for k in range(pm_cols):
            nc.tensor.matmul(
                fc2_ps[:],
                lhsT=ctx_bf[:, k:k + 1],
                rhs=w2_sb[:, k * m_shard:(k + 1) * m_shard],
                start=False, stop=(k == pm_cols - 1),
            )
        av_row = sb.tile([1, m_shard], F32, name="av_row")
        nc.scalar.activation(av_row[:], fc2_ps[:], AF.Tanh)
        nc.sync.dma_start(out_av.ap().rearrange("(p f) -> p f", p=1), av_row[:])

    nc.compile()
    return nc


def shard_inputs(hidden_states, W1, W2, n_cores=N_CORES):
    bf = ml_dtypes.bfloat16
    s, h = hidden_states.shape
    s_sh = s // n_cores
    m_sh = h // n_cores
    jt = (h // n_cores) // 128
    ht = np.ascontiguousarray(hidden_states[-1])          # [h] f32
    ht_pm = np.ascontiguousarray(ht.reshape(h // 128, 128).T).astype(bf)
    in_maps = []
    for c in range(n_cores):
        rows = hidden_states[c * s_sh:(c + 1) * s_sh]
        hid_t = np.ascontiguousarray(rows.T).astype(bf)   # [h, s_sh]
        w1s = W1[c * m_sh:(c + 1) * m_sh, :].astype(bf)
        w2t = np.ascontiguousarray(W2[c * m_sh:(c + 1) * m_sh, :].T).astype(bf)
        ht_loc = np.ascontiguousarray(ht_pm[:, c * jt:(c + 1) * jt])
        in_maps.append({
            "hid_t": hid_t, "w1s": w1s, "w2t": w2t,
            "ht_pm": ht_pm, "ht_loc": ht_loc,
        })
    return in_maps


_GRAPH = None
TRACE = False          # set True (e.g. from test.py) to capture an NTFF profile
TMPDIR = None          # optional trace output dir
LAST_RESULTS = None    # BassKernelResults of the most recent run


def kernel(hidden_states, W1, W2):
    global _GRAPH, LAST_RESULTS
    hidden_states = np.asarray(hidden_states, dtype=np.float32)
    W1 = np.asarray(W1, dtype=np.float32)
    W2 = np.asarray(W2, dtype=np.float32)
    if _GRAPH is None:
        _GRAPH = build_graph()
    in_maps = shard_inputs(hidden_states, W1, W2)
    res = run_bass_kernel_spmd(_GRAPH, in_maps, core_ids=list(range(N_CORES)),
                               trace=TRACE, tmpdir=TMPDIR)
    LAST_RESULTS = res
    outs = res.results
    aw = np.concatenate([outs[c]["out_w"] for c in range(N_CORES)])
    av = np.concatenate([outs[c]["out_av"] for c in range(N_CORES)])
    return av.astype(np.float32), aw.astype(np.float32)


# revision 21
# speedup vs baseline: 1.0253x; 1.0253x over previous
"""Distributed Trainium2 Bass kernel for nn_Attention_72791105732731.

Reference computation (S=16384, H=4096):
    score_ = hidden @ W1.T            # [S,H]
    h_t    = hidden[-1]
    score  = score_ @ h_t             # [S]
    aw     = softmax(score)
    ctx    = hidden.T @ aw            # [H]
    av     = tanh(W2 @ concat(ctx, h_t))
    return (av, aw)

Key algebraic identity: score = hidden @ (W1.T @ h_t) — reassociation turns the
550-GFLOP fc1 matmul into two matvecs, making the problem memory-bound.

Distribution over 8 cores:
  - hidden sharded over S (2048 rows/core), host-pre-transposed to [H, 2048]
    so the score contraction (over H) sits on the partition axis for TensorE.
  - W1 sharded over rows (512/core): partial v = W1_shard.T @ h_t_shard,
    AllGather + local sum (cheaper than AllReduce at this size).
  - softmax via block-local (max, sumexp) stats + AllGather of per-core stats.
  - context partials: DVE multiplies (bf16 2x mode, fused 8 tiles/op), free-dim
    sums split between the Scalar engine (activation accum_out) and GPSIMD so
    no single engine paces the pass; AllGather + local sum for the context.
  - W2 sharded over output rows (512/core), host-pre-transposed; the h_t half
    of fc2 is accumulated into PSUM during the main pass, only the ctx half
    remains after the context AllGather.

Compute dtype bf16 (validated offline: aw absmax err ~7e-5, av ~9e-3 vs fp32
reference); all accumulations fp32 (PSUM / ACT accumulator / stats math).
"""

from contextlib import ExitStack

import ml_dtypes
import numpy as np

import concourse.bass as bass
import concourse.tile as tile
from concourse import bacc, mybir
from concourse.bass_utils import run_bass_kernel_spmd

F32 = mybir.dt.float32
BF16 = mybir.dt.bfloat16
AF = mybir.ActivationFunctionType
ALU = mybir.AluOpType

N_CORES = 8
S = 16384
H = 4096


def build_graph(n_cores=N_CORES, s_shard=S // N_CORES, h=H, blk=512,
                m_shard=H // N_CORES, act_red=16, dve_red=16, fuse=8):
    """Build the SPMD single-core Bass graph (identical on every core)."""
    nb = s_shard // blk          # score/softmax blocks per core
    ht_tiles = h // 128          # h-tiles (partition tiles)
    pm_cols = h // 128           # columns of the partition-major h_t layout
    jt = (h // n_cores) // 128   # W1 row tiles per core
    k2 = 2 * h // 128            # fc2 contraction tiles
    sub = 4                      # hidden sub-DMAs per block
    assert ht_tiles % sub == 0 and ht_tiles % fuse == 0

    nc = bacc.Bacc("TRN2", target_bir_lowering=False, debug=False,
                   num_devices=n_cores)

    # ---- I/O ----
    hid_t = nc.dram_tensor("hid_t", [h, s_shard], BF16, kind="ExternalInput")
    w1s = nc.dram_tensor("w1s", [h // n_cores, h], BF16, kind="ExternalInput")
    w2t = nc.dram_tensor("w2t", [2 * h, m_shard], BF16, kind="ExternalInput")
    ht_pm = nc.dram_tensor("ht_pm", [128, pm_cols], BF16, kind="ExternalInput")
    ht_loc = nc.dram_tensor("ht_loc", [128, jt], BF16, kind="ExternalInput")
    out_w = nc.dram_tensor("out_w", [s_shard], F32, kind="ExternalOutput")
    out_av = nc.dram_tensor("out_av", [m_shard], F32, kind="ExternalOutput")

    groups = [list(range(n_cores))]

    with tile.TileContext(nc) as tc, ExitStack() as ctx:
        dram = ctx.enter_context(tc.tile_pool(name="dram", bufs=1, space="DRAM"))
        psum = ctx.enter_context(tc.tile_pool(name="psum", bufs=1, space="PSUM"))
        psum2 = ctx.enter_context(tc.tile_pool(name="psum2", bufs=2, space="PSUM"))
        sb = ctx.enter_context(tc.tile_pool(name="sb", bufs=1))
        sb2 = ctx.enter_context(tc.tile_pool(name="sb2", bufs=2))
        hidp = ctx.enter_context(tc.tile_pool(name="hidp", bufs=2))
        w1p = ctx.enter_context(tc.tile_pool(name="w1p", bufs=1))

        ones_bf = sb.tile([1, 128], BF16, name="ones_bf")
        nc.vector.memset(ones_bf[:], 1.0)
        ones_f32 = sb.tile([1, 128], F32, name="ones_f32")
        nc.vector.memset(ones_f32[:], 1.0)

        ht_loc_sb = sb.tile([128, jt], BF16, name="ht_loc_sb")
        nc.sync.dma_start(ht_loc_sb[:], ht_loc.ap())
        ht_pm_sb = sb.tile([128, pm_cols], BF16, name="ht_pm_sb")
        nc.sync.dma_start(ht_pm_sb[:], ht_pm.ap())

        # ---- v = W1.T @ h_t (partial over this core's W1 rows) ----
        # One accumulation group per PSUM bank at a time: each v column gets
        # its own psum tile (2-buf rotation) and is drained by ACT before the
        # bank is re-used.
        w1_sbs = []
        for j in range(jt):
            w1_sb = w1p.tile([128, h], BF16, name=f"w1_sb{j}", tag=f"w1{j}")
            nc.sync.dma_start(w1_sb[:], w1s.ap()[j * 128:(j + 1) * 128, :])
            w1_sbs.append(w1_sb)
        v_sb = sb.tile([128, pm_cols], F32, name="v_sb")
        for i in range(ht_tiles):
            v_ps = psum2.tile([128, 1], F32, name="v_ps", tag="vps")
            for j in range(jt):
                nc.tensor.matmul(
                    v_ps[:],
                    lhsT=w1_sbs[j][:, i * 128:(i + 1) * 128],
                    rhs=ht_loc_sb[:, j:j + 1],
                    start=(j == 0), stop=(j == jt - 1),
                )
            nc.scalar.copy(v_sb[:, i:i + 1], v_ps[:])

        # v partial -> AllGather -> local sum over ranks
        v_bounce = dram.tile([128 * pm_cols], F32, name="v_bounce")
        v_gath = dram.tile([n_cores * 128 * pm_cols], F32, name="v_gath")
        nc.sync.dma_start(v_bounce[:].rearrange("(p t) -> p t", p=128), v_sb[:])
        nc.gpsimd.collective_compute(
            "AllGather", ALU.bypass, replica_groups=groups,
            ins=[v_bounce.opt()], outs=[v_gath.opt()],
        )
        v_all = sb.tile([128, pm_cols * n_cores], F32, name="v_all")
        nc.sync.dma_start(
            v_all[:].rearrange("p (t r) -> p t r", r=n_cores),
            v_gath[:].rearrange("(r p t) -> p t r", p=128, t=pm_cols))
        v_rd = sb.tile([128, pm_cols], F32, name="v_rd")
        nc.vector.reduce_sum(
            out=v_rd[:],
            in_=v_all[:].rearrange("p (t r) -> p t r", r=n_cores),
            axis=mybir.AxisListType.X)
        v_pm = sb.tile([128, pm_cols], BF16, name="v_pm")
        nc.scalar.copy(v_pm[:], v_rd[:])

        # ---- main pass over s-blocks: score -> exp -> context partials ----
        e_rows = sb.tile([1, s_shard], F32, name="e_rows")
        mb_row = sb.tile([1, nb], F32, name="mb_row")
        negmb_row = sb.tile([1, nb], F32, name="negmb_row")
        zb_row = sb.tile([1, nb], F32, name="zb_row")
        ctx_store = sb.tile([128, ht_tiles * nb], F32, name="ctx_store")

        # Phase A (high priority): per block, DMA + score + exp + e-broadcast.
        # Phase B (lower priority): the context multiply/reduce bulk. Emitting
        # all of A before B keeps the score->exp->broadcast critical chain
        # ahead of the reduce backlog in every engine queue.
        hid_sbs = []
        e_bcs = []
        for b in range(nb):
            hid_sb = hidp.tile([128, ht_tiles * blk], BF16, name="hid_sb",
                               tag="hid")
            hid_sbs.append(hid_sb)
            for g in range(sub):
                tpg = ht_tiles // sub  # h-tiles per sub-DMA
                nc.sync.dma_start(
                    hid_sb[:, g * tpg * blk:(g + 1) * tpg * blk]
                    .rearrange("p (t s) -> p t s", t=tpg),
                    hid_t.ap()[g * tpg * 128:(g + 1) * tpg * 128,
                               b * blk:(b + 1) * blk]
                    .rearrange("(t p) s -> p t s", p=128),
                )

            score_ps = psum2.tile([1, blk], F32, name="score_ps", tag="score")
            for t in range(ht_tiles):
                nc.tensor.matmul(
                    score_ps[:],
                    lhsT=v_pm[:, t:t + 1],
                    rhs=hid_sb[:, t * blk:(t + 1) * blk],
                    start=(t == 0), stop=(t == ht_tiles - 1),
                )

            nc.vector.reduce_max(out=mb_row[:, b:b + 1], in_=score_ps[:],
                                 axis=mybir.AxisListType.X)
            nc.scalar.mul(negmb_row[:, b:b + 1], mb_row[:, b:b + 1], -1.0)
            # e = exp(score - m_b); Z_b accumulated by the activation engine
            nc.scalar.activation(
                e_rows[:, b * blk:(b + 1) * blk], score_ps[:], AF.Exp,
                bias=negmb_row[:, b:b + 1], scale=1.0,
                accum_out=zb_row[:, b:b + 1],
            )
            # broadcast e across partitions via PE (bf16), cast to bf16
            e_bf = sb2.tile([1, blk], BF16, name="e_bf", tag="ebf")
            nc.scalar.copy(e_bf[:], e_rows[:, b * blk:(b + 1) * blk])
            e_bc_ps = psum2.tile([128, blk], F32, name="e_bc_ps", tag="ebc")
            nc.tensor.matmul(e_bc_ps[:], lhsT=ones_bf[:], rhs=e_bf[:],
                             start=True, stop=True)
            e_bc = sb2.tile([128, blk], BF16, name="e_bc", tag="ebc_sb",
                            bufs=nb)
            nc.scalar.copy(e_bc[:], e_bc_ps[:])
            e_bcs.append(e_bc)

        # ---- local stats -> AllGather (overlaps phase B below) ----
        neg_m_core = sb.tile([1, 1], F32, name="neg_m_core")
        nc.vector.tensor_reduce(out=neg_m_core[:], in_=mb_row[:],
                                axis=mybir.AxisListType.X, op=ALU.max,
                                negate=True)
        alpha = sb.tile([1, nb], F32, name="alpha")
        nc.scalar.activation(alpha[:], negmb_row[:], AF.Exp,
                             bias=neg_m_core[:], scale=-1.0)
        scr_nb = sb.tile([1, nb], F32, name="scr_nb")
        z_core = sb.tile([1, 1], F32, name="z_core")
        nc.vector.tensor_mul(scr_nb[:], alpha[:], zb_row[:])
        nc.vector.reduce_sum(out=z_core[:], in_=scr_nb[:],
                             axis=mybir.AxisListType.X)
        stats_sb = sb.tile([1, 2], F32, name="stats_sb")
        nc.scalar.mul(stats_sb[:, 0:1], neg_m_core[:], -1.0)
        nc.scalar.copy(stats_sb[:, 1:2], z_core[:])
        stats_bounce = dram.tile([2], F32, name="stats_bounce")
        stats_all = dram.tile([2 * n_cores], F32, name="stats_all")
        nc.sync.dma_start(stats_bounce[:].rearrange("(p f) -> p f", p=1),
                          stats_sb[:])
        nc.gpsimd.collective_compute(
            "AllGather", ALU.bypass, replica_groups=groups,
            ins=[stats_bounce.opt()], outs=[stats_all.opt()],
        )

        # ---- Phase B: context partials ctx_store[:, b*HT + t] = sum_s hid*e
        # DVE does fused multiplies; per-tile free-dim sums split ACT/DVE.
        for b in range(nb):
            hid_sb = hid_sbs[b]
            e_bc = e_bcs[b]
            e_rep = (e_bc[:].rearrange("p (o s) -> p o s", o=1)
                     .broadcast_to((128, fuse, blk)))
            for f in range(ht_tiles // fuse):
                tt_out = sb2.tile([128, fuse * blk], BF16, name="tt_out",
                                  tag="tt", bufs=2)
                nc.vector.tensor_mul(
                    tt_out[:].rearrange("p (t s) -> p t s", t=fuse),
                    hid_sb[:, f * fuse * blk:(f + 1) * fuse * blk]
                    .rearrange("p (t s) -> p t s", t=fuse),
                    e_rep)
                for ti in range(fuse):
                    t = f * fuse + ti
                    col = ctx_store[:, b * ht_tiles + t:b * ht_tiles + t + 1]
                    src = tt_out[:, ti * blk:(ti + 1) * blk]
                    if t < act_red:
                        junk_ps = psum2.tile([128, blk], F32, name="junk_ps",
                                             tag="junk", bufs=1)
                        nc.scalar.activation(junk_ps[:], src, AF.Copy,
                                             accum_out=col)
                    else:
                        nc.vector.reduce_sum(out=col, in_=src,
                                             axis=mybir.AxisListType.X)

        # ---- fc2 h_t half: runs on PE while phase B / collectives proceed
        w2_sb = sb.tile([128, k2 * m_shard], BF16, name="w2_sb")
        w2_sub = 8
        tpg2 = k2 // w2_sub
        for g in range(w2_sub):
            nc.sync.dma_start(
                w2_sb[:, g * tpg2 * m_shard:(g + 1) * tpg2 * m_shard]
                .rearrange("p (t m) -> p t m", t=tpg2),
                w2t.ap()[g * tpg2 * 128:(g + 1) * tpg2 * 128, :]
                .rearrange("(t p) m -> p t m", p=128),
            )
        fc2_ps = psum.tile([1, m_shard], F32, name="fc2_ps")
        for k in range(pm_cols, k2):
            nc.tensor.matmul(
                fc2_ps[:],
                lhsT=ht_pm_sb[:, k - pm_cols:k - pm_cols + 1],
                rhs=w2_sb[:, k * m_shard:(k + 1) * m_shard],
                start=(k == pm_cols), stop=False,
            )

        # ---- global softmax factors from gathered stats ----
        m_all = sb.tile([1, n_cores], F32, name="m_all")
        z_all = sb.tile([1, n_cores], F32, name="z_all")
        strided = stats_all[:].rearrange("(r two) -> two r", two=2)
        nc.sync.dma_start(m_all[:], strided[0:1, :])
        nc.sync.dma_start(z_all[:], strided[1:2, :])

        neg_m_g = sb.tile([1, 1], F32, name="neg_m_g")
        nc.vector.tensor_reduce(out=neg_m_g[:], in_=m_all[:],
                                axis=mybir.AxisListType.X, op=ALU.max,
                                negate=True)
        beta = sb.tile([1, n_cores], F32, name="beta")
        nc.scalar.activation(beta[:], m_all[:], AF.Exp, bias=neg_m_g[:],
                             scale=1.0)
        scr_nc = sb.tile([1, n_cores], F32, name="scr_nc")
        z_g = sb.tile([1, 1], F32, name="z_g")
        nc.vector.tensor_mul(scr_nc[:], beta[:], z_all[:])
        nc.vector.reduce_sum(out=z_g[:], in_=scr_nc[:],
                             axis=mybir.AxisListType.X)
        inv_zg = sb.tile([1, 1], F32, name="inv_zg")
        nc.vector.reciprocal(inv_zg[:], z_g[:])
        # gamma_b = exp(m_b - m_g) / Z_g
        gamma = sb.tile([1, nb], F32, name="gamma")
        nc.scalar.activation(gamma[:], negmb_row[:], AF.Exp, bias=neg_m_g[:],
                             scale=-1.0)
        gamma2 = sb.tile([1, nb], F32, name="gamma2")
        nc.vector.tensor_scalar_mul(gamma2[:], gamma[:], inv_zg[:])

        # ---- attention weights output ----
        w_row = sb.tile([1, s_shard], F32, name="w_row")
        for b in range(nb):
            nc.vector.tensor_scalar_mul(w_row[:, b * blk:(b + 1) * blk],
                                        e_rows[:, b * blk:(b + 1) * blk],
                                        gamma2[:, b:b + 1])
        nc.sync.dma_start(out_w.ap().rearrange("(p f) -> p f", p=1), w_row[:])

        # ---- combine context partials, AllGather + local sum ----
        gam_ps = psum2.tile([128, nb], F32, name="gam_ps", tag="vps")
        nc.tensor.matmul(gam_ps[:], lhsT=ones_f32[:], rhs=gamma2[:],
                         start=True, stop=True)
        gam_sb = sb.tile([128, nb], F32, name="gam_sb")
        nc.scalar.copy(gam_sb[:], gam_ps[:])

        ctx_acc = sb.tile([128, ht_tiles], F32, name="ctx_acc")
        ctx_tmp = sb.tile([128, ht_tiles], F32, name="ctx_tmp")
        nc.vector.tensor_scalar_mul(
            ctx_acc[:], ctx_store[:, 0:ht_tiles], gam_sb[:, 0:1])
        for b in range(1, nb):
            nc.vector.tensor_scalar_mul(
                ctx_tmp[:], ctx_store[:, b * ht_tiles:(b + 1) * ht_tiles],
                gam_sb[:, b:b + 1])
            nc.vector.tensor_add(ctx_acc[:], ctx_acc[:], ctx_tmp[:])

        ctx_bounce = dram.tile([h], F32, name="ctx_bounce")
        ctx_gath = dram.tile([n_cores * h], F32, name="ctx_gath")
        nc.sync.dma_start(ctx_bounce[:].rearrange("(p t) -> p t", p=128),
                          ctx_acc[:])
        nc.gpsimd.collective_compute(
            "AllGather", ALU.bypass, replica_groups=groups,
            ins=[ctx_bounce.opt()], outs=[ctx_gath.opt()],
        )
        ctx_all = sb.tile([128, ht_tiles * n_cores], F32, name="ctx_all")
        nc.sync.dma_start(
            ctx_all[:].rearrange("p (t r) -> p t r", r=n_cores),
            ctx_gath[:].rearrange("(r p t) -> p t r", p=128, t=ht_tiles))
        ctx_rd = sb.tile([128, ht_tiles], F32, name="ctx_rd")
        nc.vector.reduce_sum(
            out=ctx_rd[:],
            in_=ctx_all[:].rearrange("p (t r) -> p t r", r=n_cores),
            axis=mybir.AxisListType.X)

        # ---- fc2 ctx half + tanh ----
        ctx_bf = sb.tile([128, ht_tiles], BF16, name="ctx_bf")
        nc.scalar.copy(ctx_bf[:], ctx_rd[:])
        for k in range(pm_cols):
            nc.tensor.matmul(
                fc2_ps[:],
                lhsT=ctx_bf[:, k:k + 1],
                rhs=w2_sb[:, k * m_shard:(k + 1) * m_shard],
                start=False, stop=(k == pm_cols - 1),
            )
        av_row = sb.tile([1, m_shard], F32, name="av_row")
        nc.scalar.activation(av_row[:], fc2_ps[:], AF.Tanh)
        nc.sync.dma_start(out_av.ap().rearrange("(p f) -> p f", p=1), av_row[:])

    nc.compile()
    return nc


def shard_inputs(hidden_states, W1, W2, n_cores=N_CORES):
    bf = ml_dtypes.bfloat16
    s, h = hidden_states.shape
    s_sh = s // n_cores
    m_sh = h // n_cores
    jt = (h // n_cores) // 128
    ht = np.ascontiguousarray(hidden_states[-1])          # [h] f32
    ht_pm = np.ascontiguousarray(ht.reshape(h // 128, 128).T).astype(bf)
    in_maps = []
    for c in range(n_cores):
        rows = hidden_states[c * s_sh:(c + 1) * s_sh]
        hid_t = np.ascontiguousarray(rows.T).astype(bf)   # [h, s_sh]
        w1s = W1[c * m_sh:(c + 1) * m_sh, :].astype(bf)
        w2t = np.ascontiguousarray(W2[c * m_sh:(c + 1) * m_sh, :].T).astype(bf)
        ht_loc = np.ascontiguousarray(ht_pm[:, c * jt:(c + 1) * jt])
        in_maps.append({
            "hid_t": hid_t, "w1s": w1s, "w2t": w2t,
            "ht_pm": ht_pm, "ht_loc": ht_loc,
        })
    return in_maps


_GRAPH = None
TRACE = False          # set True (e.g. from test.py) to capture an NTFF profile
TMPDIR = None          # optional trace output dir
LAST_RESULTS = None    # BassKernelResults of the most recent run


def kernel(hidden_states, W1, W2):
    global _GRAPH, LAST_RESULTS
    hidden_states = np.asarray(hidden_states, dtype=np.float32)
    W1 = np.asarray(W1, dtype=np.float32)
    W2 = np.asarray(W2, dtype=np.float32)
    if _GRAPH is None:
        _GRAPH = build_graph()
    in_maps = shard_inputs(hidden_states, W1, W2)
    res = run_bass_kernel_spmd(_GRAPH, in_maps, core_ids=list(range(N_CORES)),
                               trace=TRACE, tmpdir=TMPDIR)
    LAST_RESULTS = res
    outs = res.results
    aw = np.concatenate([outs[c]["out_w"] for c in range(N_CORES)])
    av = np.concatenate([outs[c]["out_av"] for c in range(N_CORES)])
    return av.astype(np.float32), aw.astype(np.float32)


# revision 23
# speedup vs baseline: 1.0479x; 1.0221x over previous
"""Distributed Trainium2 Bass kernel for nn_Attention_72791105732731.

Reference computation (S=16384, H=4096):
    score_ = hidden @ W1.T            # [S,H]
    h_t    = hidden[-1]
    score  = score_ @ h_t             # [S]
    aw     = softmax(score)
    ctx    = hidden.T @ aw            # [H]
    av     = tanh(W2 @ concat(ctx, h_t))
    return (av, aw)

Key algebraic identity: score = hidden @ (W1.T @ h_t) — reassociation turns the
550-GFLOP fc1 matmul into two matvecs, making the problem memory-bound.

Distribution over 8 cores:
  - hidden sharded over S (2048 rows/core), host-pre-transposed to [H, 2048]
    so the score contraction (over H) sits on the partition axis for TensorE.
  - W1 sharded over rows (512/core): partial v = W1_shard.T @ h_t_shard,
    AllGather + local sum (cheaper than AllReduce at this size).
  - softmax via block-local (max, sumexp) stats + AllGather of per-core stats.
  - context partials: DVE multiplies (bf16 2x mode, fused 8 tiles/op), free-dim
    sums split between the Scalar engine (activation accum_out) and GPSIMD so
    no single engine paces the pass; AllGather + local sum for the context.
  - W2 sharded over output rows (512/core), host-pre-transposed; the h_t half
    of fc2 is accumulated into PSUM during the main pass, only the ctx half
    remains after the context AllGather.

Compute dtype bf16 (validated offline: aw absmax err ~7e-5, av ~9e-3 vs fp32
reference); all accumulations fp32 (PSUM / ACT accumulator / stats math).
"""

from contextlib import ExitStack

import ml_dtypes
import numpy as np

import concourse.bass as bass
import concourse.tile as tile
from concourse import bacc, mybir
from concourse.bass_utils import run_bass_kernel_spmd

F32 = mybir.dt.float32
BF16 = mybir.dt.bfloat16
AF = mybir.ActivationFunctionType
ALU = mybir.AluOpType

N_CORES = 8
S = 16384
H = 4096


def build_graph(n_cores=N_CORES, s_shard=S // N_CORES, h=H, blk=512,
                m_shard=H // N_CORES, act_red=16, dve_red=16, fuse=8):
    """Build the SPMD single-core Bass graph (identical on every core)."""
    nb = s_shard // blk          # score/softmax blocks per core
    ht_tiles = h // 128          # h-tiles (partition tiles)
    pm_cols = h // 128           # columns of the partition-major h_t layout
    jt = (h // n_cores) // 128   # W1 row tiles per core
    k2 = 2 * h // 128            # fc2 contraction tiles
    sub = 4                      # hidden sub-DMAs per block
    assert ht_tiles % sub == 0 and ht_tiles % fuse == 0

    nc = bacc.Bacc("TRN2", target_bir_lowering=False, debug=False,
                   num_devices=n_cores)

    # ---- I/O ----
    hid_t = nc.dram_tensor("hid_t", [h, s_shard], BF16, kind="ExternalInput")
    w1s = nc.dram_tensor("w1s", [h // n_cores, h], BF16, kind="ExternalInput")
    w2t = nc.dram_tensor("w2t", [2 * h, m_shard], BF16, kind="ExternalInput")
    ht_pm = nc.dram_tensor("ht_pm", [128, pm_cols], BF16, kind="ExternalInput")
    ht_loc = nc.dram_tensor("ht_loc", [128, jt], BF16, kind="ExternalInput")
    out_w = nc.dram_tensor("out_w", [s_shard], F32, kind="ExternalOutput")
    out_av = nc.dram_tensor("out_av", [m_shard], F32, kind="ExternalOutput")

    groups = [list(range(n_cores))]

    with tile.TileContext(nc) as tc, ExitStack() as ctx:
        dram = ctx.enter_context(tc.tile_pool(name="dram", bufs=1, space="DRAM"))
        psum = ctx.enter_context(tc.tile_pool(name="psum", bufs=1, space="PSUM"))
        psum2 = ctx.enter_context(tc.tile_pool(name="psum2", bufs=2, space="PSUM"))
        sb = ctx.enter_context(tc.tile_pool(name="sb", bufs=1))
        sb2 = ctx.enter_context(tc.tile_pool(name="sb2", bufs=2))
        hidp = ctx.enter_context(tc.tile_pool(name="hidp", bufs=2))
        w1p = ctx.enter_context(tc.tile_pool(name="w1p", bufs=1))

        ones_bf = sb.tile([1, 128], BF16, name="ones_bf")
        nc.vector.memset(ones_bf[:], 1.0)
        ones_f32 = sb.tile([1, 128], F32, name="ones_f32")
        nc.vector.memset(ones_f32[:], 1.0)

        ht_loc_sb = sb.tile([128, jt], BF16, name="ht_loc_sb")
        nc.sync.dma_start(ht_loc_sb[:], ht_loc.ap())
        ht_pm_sb = sb.tile([128, pm_cols], BF16, name="ht_pm_sb")
        nc.sync.dma_start(ht_pm_sb[:], ht_pm.ap())

        # Warm-up collective: the first collective of a NEFF absorbs the
        # cross-core launch skew + firmware warm-up (measured 25-50us). Fire a
        # tiny AllGather immediately so the real collectives run at the
        # few-microsecond floor. Reuses the stats buffers (they are
        # overwritten before the real stats AllGather).
        stats_bounce = dram.tile([2], F32, name="stats_bounce")
        stats_all = dram.tile([2 * n_cores], F32, name="stats_all")
        warm_sb = sb.tile([1, 2], F32, name="warm_sb")
        nc.vector.memset(warm_sb[:], 0.0)
        nc.sync.dma_start(stats_bounce[:].rearrange("(p f) -> p f", p=1),
                          warm_sb[:])
        nc.gpsimd.collective_compute(
            "AllGather", ALU.bypass, replica_groups=groups,
            ins=[stats_bounce.opt()], outs=[stats_all.opt()],
        )

        # ---- v = W1.T @ h_t (partial over this core's W1 rows) ----
        # One accumulation group per PSUM bank at a time: each v column gets
        # its own psum tile (2-buf rotation) and is drained by ACT before the
        # bank is re-used.
        w1_sbs = []
        for j in range(jt):
            w1_sb = w1p.tile([128, h], BF16, name=f"w1_sb{j}", tag=f"w1{j}")
            nc.sync.dma_start(w1_sb[:], w1s.ap()[j * 128:(j + 1) * 128, :])
            w1_sbs.append(w1_sb)
        v_sb = sb.tile([128, pm_cols], F32, name="v_sb")
        for i in range(ht_tiles):
            v_ps = psum2.tile([128, 1], F32, name="v_ps", tag="vps")
            for j in range(jt):
                nc.tensor.matmul(
                    v_ps[:],
                    lhsT=w1_sbs[j][:, i * 128:(i + 1) * 128],
                    rhs=ht_loc_sb[:, j:j + 1],
                    start=(j == 0), stop=(j == jt - 1),
                )
            nc.scalar.copy(v_sb[:, i:i + 1], v_ps[:])

        # v partial -> AllGather -> local sum over ranks
        v_bounce = dram.tile([128 * pm_cols], F32, name="v_bounce")
        v_gath = dram.tile([n_cores * 128 * pm_cols], F32, name="v_gath")
        nc.sync.dma_start(v_bounce[:].rearrange("(p t) -> p t", p=128), v_sb[:])
        nc.gpsimd.collective_compute(
            "AllGather", ALU.bypass, replica_groups=groups,
            ins=[v_bounce.opt()], outs=[v_gath.opt()],
        )
        v_all = sb.tile([128, pm_cols * n_cores], F32, name="v_all")
        nc.sync.dma_start(
            v_all[:].rearrange("p (t r) -> p t r", r=n_cores),
            v_gath[:].rearrange("(r p t) -> p t r", p=128, t=pm_cols))
        v_rd = sb.tile([128, pm_cols], F32, name="v_rd")
        nc.vector.reduce_sum(
            out=v_rd[:],
            in_=v_all[:].rearrange("p (t r) -> p t r", r=n_cores),
            axis=mybir.AxisListType.X)
        v_pm = sb.tile([128, pm_cols], BF16, name="v_pm")
        nc.scalar.copy(v_pm[:], v_rd[:])

        # ---- main pass over s-blocks: score -> exp -> context partials ----
        e_rows = sb.tile([1, s_shard], BF16, name="e_rows")
        mb_row = sb.tile([1, nb], F32, name="mb_row")
        negmb_row = sb.tile([1, nb], F32, name="negmb_row")
        zb_row = sb.tile([1, nb], F32, name="zb_row")
        ctx_store = sb.tile([128, ht_tiles * nb], F32, name="ctx_store")

        for b in range(nb):
            hid_sb = hidp.tile([128, ht_tiles * blk], BF16, name="hid_sb",
                               tag="hid")
            for g in range(sub):
                tpg = ht_tiles // sub  # h-tiles per sub-DMA
                nc.sync.dma_start(
                    hid_sb[:, g * tpg * blk:(g + 1) * tpg * blk]
                    .rearrange("p (t s) -> p t s", t=tpg),
                    hid_t.ap()[g * tpg * 128:(g + 1) * tpg * 128,
                               b * blk:(b + 1) * blk]
                    .rearrange("(t p) s -> p t s", p=128),
                )

            score_ps = psum2.tile([1, blk], F32, name="score_ps", tag="score")
            for t in range(ht_tiles):
                nc.tensor.matmul(
                    score_ps[:],
                    lhsT=v_pm[:, t:t + 1],
                    rhs=hid_sb[:, t * blk:(t + 1) * blk],
                    start=(t == 0), stop=(t == ht_tiles - 1),
                )

            nc.vector.reduce_max(out=mb_row[:, b:b + 1], in_=score_ps[:],
                                 axis=mybir.AxisListType.X)
            nc.scalar.mul(negmb_row[:, b:b + 1], mb_row[:, b:b + 1], -1.0)
            # e = exp(score - m_b) straight to bf16; Z_b via the ACT
            # accumulator (fp32)
            nc.scalar.activation(
                e_rows[:, b * blk:(b + 1) * blk], score_ps[:], AF.Exp,
                bias=negmb_row[:, b:b + 1], scale=1.0,
                accum_out=zb_row[:, b:b + 1],
            )
            # broadcast e across partitions via PE (bf16)
            e_bc_ps = psum2.tile([128, blk], F32, name="e_bc_ps", tag="ebc")
            nc.tensor.matmul(e_bc_ps[:], lhsT=ones_bf[:],
                             rhs=e_rows[:, b * blk:(b + 1) * blk],
                             start=True, stop=True)
            e_bc = sb2.tile([128, blk], BF16, name="e_bc", tag="ebc_sb")
            nc.scalar.copy(e_bc[:], e_bc_ps[:])

            if b == nb - 1:
                # local stats + stats AllGather — emitted before the trailing
                # context reduces so the collective fires as soon as the last
                # score block is done
                neg_m_core = sb.tile([1, 1], F32, name="neg_m_core")
                nc.vector.tensor_reduce(out=neg_m_core[:], in_=mb_row[:],
                                        axis=mybir.AxisListType.X, op=ALU.max,
                                        negate=True)
                alpha = sb.tile([1, nb], F32, name="alpha")
                nc.scalar.activation(alpha[:], negmb_row[:], AF.Exp,
                                     bias=neg_m_core[:], scale=-1.0)
                scr_nb = sb.tile([1, nb], F32, name="scr_nb")
                z_core = sb.tile([1, 1], F32, name="z_core")
                nc.vector.tensor_mul(scr_nb[:], alpha[:], zb_row[:])
                nc.vector.reduce_sum(out=z_core[:], in_=scr_nb[:],
                                     axis=mybir.AxisListType.X)
                stats_sb = sb.tile([1, 2], F32, name="stats_sb")
                nc.scalar.mul(stats_sb[:, 0:1], neg_m_core[:], -1.0)
                nc.scalar.copy(stats_sb[:, 1:2], z_core[:])
                nc.sync.dma_start(
                    stats_bounce[:].rearrange("(p f) -> p f", p=1),
                    stats_sb[:])
                nc.gpsimd.collective_compute(
                    "AllGather", ALU.bypass, replica_groups=groups,
                    ins=[stats_bounce.opt()], outs=[stats_all.opt()],
                )

            # context partials: ctx_store[:, b*HT + t] = sum_s hid*e
            # DVE does fused multiplies; per-tile free-dim sums split ACT/DVE.
            e_rep = (e_bc[:].rearrange("p (o s) -> p o s", o=1)
                     .broadcast_to((128, fuse, blk)))
            for f in range(ht_tiles // fuse):
                tt_out = sb2.tile([128, fuse * blk], BF16, name="tt_out",
                                  tag="tt", bufs=2)
                nc.vector.tensor_mul(
                    tt_out[:].rearrange("p (t s) -> p t s", t=fuse),
                    hid_sb[:, f * fuse * blk:(f + 1) * fuse * blk]
                    .rearrange("p (t s) -> p t s", t=fuse),
                    e_rep)
                for ti in range(fuse):
                    t = f * fuse + ti
                    col = ctx_store[:, b * ht_tiles + t:b * ht_tiles + t + 1]
                    src = tt_out[:, ti * blk:(ti + 1) * blk]
                    if t < act_red:
                        junk_ps = psum2.tile([128, blk], F32, name="junk_ps",
                                             tag="junk", bufs=1)
                        nc.scalar.activation(junk_ps[:], src, AF.Copy,
                                             accum_out=col)
                    else:
                        nc.vector.reduce_sum(out=col, in_=src,
                                             axis=mybir.AxisListType.X)

        # ---- fc2 h_t half: runs on PE while phase B / collectives proceed
        w2_sb = sb.tile([128, k2 * m_shard], BF16, name="w2_sb")
        w2_sub = 8
        tpg2 = k2 // w2_sub
        for g in range(w2_sub):
            nc.sync.dma_start(
                w2_sb[:, g * tpg2 * m_shard:(g + 1) * tpg2 * m_shard]
                .rearrange("p (t m) -> p t m", t=tpg2),
                w2t.ap()[g * tpg2 * 128:(g + 1) * tpg2 * 128, :]
                .rearrange("(t p) m -> p t m", p=128),
            )
        fc2_ps = psum.tile([1, m_shard], F32, name="fc2_ps")
        for k in range(pm_cols, k2):
            nc.tensor.matmul(
                fc2_ps[:],
                lhsT=ht_pm_sb[:, k - pm_cols:k - pm_cols + 1],
                rhs=w2_sb[:, k * m_shard:(k + 1) * m_shard],
                start=(k == pm_cols), stop=False,
            )

        # ---- global softmax factors from gathered stats ----
        m_all = sb.tile([1, n_cores], F32, name="m_all")
        z_all = sb.tile([1, n_cores], F32, name="z_all")
        strided = stats_all[:].rearrange("(r two) -> two r", two=2)
        nc.sync.dma_start(m_all[:], strided[0:1, :])
        nc.sync.dma_start(z_all[:], strided[1:2, :])

        neg_m_g = sb.tile([1, 1], F32, name="neg_m_g")
        nc.vector.tensor_reduce(out=neg_m_g[:], in_=m_all[:],
                                axis=mybir.AxisListType.X, op=ALU.max,
                                negate=True)
        beta = sb.tile([1, n_cores], F32, name="beta")
        nc.scalar.activation(beta[:], m_all[:], AF.Exp, bias=neg_m_g[:],
                             scale=1.0)
        scr_nc = sb.tile([1, n_cores], F32, name="scr_nc")
        z_g = sb.tile([1, 1], F32, name="z_g")
        nc.vector.tensor_mul(scr_nc[:], beta[:], z_all[:])
        nc.vector.reduce_sum(out=z_g[:], in_=scr_nc[:],
                             axis=mybir.AxisListType.X)
        inv_zg = sb.tile([1, 1], F32, name="inv_zg")
        nc.vector.reciprocal(inv_zg[:], z_g[:])
        # gamma_b = exp(m_b - m_g) / Z_g
        gamma = sb.tile([1, nb], F32, name="gamma")
        nc.scalar.activation(gamma[:], negmb_row[:], AF.Exp, bias=neg_m_g[:],
                             scale=-1.0)
        gamma2 = sb.tile([1, nb], F32, name="gamma2")
        nc.vector.tensor_scalar_mul(gamma2[:], gamma[:], inv_zg[:])

        # ---- attention weights output ----
        w_row = sb.tile([1, s_shard], F32, name="w_row")
        for b in range(nb):
            nc.vector.tensor_scalar_mul(w_row[:, b * blk:(b + 1) * blk],
                                        e_rows[:, b * blk:(b + 1) * blk],
                                        gamma2[:, b:b + 1])
        nc.sync.dma_start(out_w.ap().rearrange("(p f) -> p f", p=1), w_row[:])

        # ---- combine context partials, AllGather + local sum ----
        gam_ps = psum2.tile([128, nb], F32, name="gam_ps", tag="vps")
        nc.tensor.matmul(gam_ps[:], lhsT=ones_f32[:], rhs=gamma2[:],
                         start=True, stop=True)
        gam_sb = sb.tile([128, nb], F32, name="gam_sb")
        nc.scalar.copy(gam_sb[:], gam_ps[:])

        ctx_acc = sb.tile([128, ht_tiles], F32, name="ctx_acc")
        ctx_tmp = sb.tile([128, ht_tiles], F32, name="ctx_tmp")
        nc.vector.tensor_scalar_mul(
            ctx_acc[:], ctx_store[:, 0:ht_tiles], gam_sb[:, 0:1])
        for b in range(1, nb):
            nc.vector.tensor_scalar_mul(
                ctx_tmp[:], ctx_store[:, b * ht_tiles:(b + 1) * ht_tiles],
                gam_sb[:, b:b + 1])
            nc.vector.tensor_add(ctx_acc[:], ctx_acc[:], ctx_tmp[:])

        ctx_bounce = dram.tile([h], F32, name="ctx_bounce")
        ctx_gath = dram.tile([n_cores * h], F32, name="ctx_gath")
        nc.sync.dma_start(ctx_bounce[:].rearrange("(p t) -> p t", p=128),
                          ctx_acc[:])
        nc.gpsimd.collective_compute(
            "AllGather", ALU.bypass, replica_groups=groups,
            ins=[ctx_bounce.opt()], outs=[ctx_gath.opt()],
        )
        ctx_all = sb.tile([128, ht_tiles * n_cores], F32, name="ctx_all")
        nc.sync.dma_start(
            ctx_all[:].rearrange("p (t r) -> p t r", r=n_cores),
            ctx_gath[:].rearrange("(r p t) -> p t r", p=128, t=ht_tiles))
        ctx_rd = sb.tile([128, ht_tiles], F32, name="ctx_rd")
        nc.vector.reduce_sum(
            out=ctx_rd[:],
            in_=ctx_all[:].rearrange("p (t r) -> p t r", r=n_cores),
            axis=mybir.AxisListType.X)

        # ---- fc2 ctx half + tanh ----
        ctx_bf = sb.tile([128, ht_tiles], BF16, name="ctx_bf")
        nc.scalar.copy(ctx_bf[:], ctx_rd[:])
        for k in range(pm_cols):
            nc.tensor.matmul(
                fc2_ps[:],
                lhsT=ctx_bf[:, k:k + 1],
                rhs=w2_sb[:, k * m_shard:(k + 1) * m_shard],
                start=False, stop=(k == pm_cols - 1),
            )
        av_row = sb.tile([1, m_shard], F32, name="av_row")
        nc.scalar.activation(av_row[:], fc2_ps[:], AF.Tanh)
        nc.sync.dma_start(out_av.ap().rearrange("(p f) -> p f", p=1), av_row[:])

    nc.compile()
    return nc


def shard_inputs(hidden_states, W1, W2, n_cores=N_CORES):
    bf = ml_dtypes.bfloat16
    s, h = hidden_states.shape
    s_sh = s // n_cores
    m_sh = h // n_cores
    jt = (h // n_cores) // 128
    ht = np.ascontiguousarray(hidden_states[-1])          # [h] f32
    ht_pm = np.ascontiguousarray(ht.reshape(h // 128, 128).T).astype(bf)
    in_maps = []
    for c in range(n_cores):
        rows = hidden_states[c * s_sh:(c + 1) * s_sh]
        hid_t = np.ascontiguousarray(rows.T).astype(bf)   # [h, s_sh]
        w1s = W1[c * m_sh:(c + 1) * m_sh, :].astype(bf)
        w2t = np.ascontiguousarray(W2[c * m_sh:(c + 1) * m_sh, :].T).astype(bf)
        ht_loc = np.ascontiguousarray(ht_pm[:, c * jt:(c + 1) * jt])
        in_maps.append({
            "hid_t": hid_t, "w1s": w1s, "w2t": w2t,
            "ht_pm": ht_pm, "ht_loc": ht_loc,
        })
    return in_maps


_GRAPH = None
TRACE = False          # set True (e.g. from test.py) to capture an NTFF profile
TMPDIR = None          # optional trace output dir
LAST_RESULTS = None    # BassKernelResults of the most recent run


def kernel(hidden_states, W1, W2):
    global _GRAPH, LAST_RESULTS
    hidden_states = np.asarray(hidden_states, dtype=np.float32)
    W1 = np.asarray(W1, dtype=np.float32)
    W2 = np.asarray(W2, dtype=np.float32)
    if _GRAPH is None:
        _GRAPH = build_graph()
    in_maps = shard_inputs(hidden_states, W1, W2)
    res = run_bass_kernel_spmd(_GRAPH, in_maps, core_ids=list(range(N_CORES)),
                               trace=TRACE, tmpdir=TMPDIR)
    LAST_RESULTS = res
    outs = res.results
    aw = np.concatenate([outs[c]["out_w"] for c in range(N_CORES)])
    av = np.concatenate([outs[c]["out_av"] for c in range(N_CORES)])
    return av.astype(np.float32), aw.astype(np.float32)


# revision 25
# speedup vs baseline: 1.3418x; 1.2805x over previous
"""Distributed Trainium2 Bass kernel for nn_Attention_72791105732731.

Reference computation (S=16384, H=4096):
    score_ = hidden @ W1.T            # [S,H]
    h_t    = hidden[-1]
    score  = score_ @ h_t             # [S]
    aw     = softmax(score)
    ctx    = hidden.T @ aw            # [H]
    av     = tanh(W2 @ concat(ctx, h_t))
    return (av, aw)

Key algebraic identity: score = hidden @ (W1.T @ h_t) — reassociation turns the
550-GFLOP fc1 matmul into two matvecs, making the problem memory-bound.

Distribution over 8 cores:
  - hidden sharded over S (2048 rows/core), host-pre-transposed to [H, 2048]
    so the score contraction (over H) sits on the partition axis for TensorE.
  - W1 sharded over rows (512/core): partial v = W1_shard.T @ h_t_shard,
    AllGather + local sum (cheaper than AllReduce at this size).
  - softmax via block-local (max, sumexp) stats + AllGather of per-core stats.
  - context partials: DVE multiplies (bf16 2x mode, fused 8 tiles/op), free-dim
    sums split between the Scalar engine (activation accum_out) and GPSIMD so
    no single engine paces the pass; AllGather + local sum for the context.
  - W2 sharded over output rows (512/core), host-pre-transposed; the h_t half
    of fc2 is accumulated into PSUM during the main pass, only the ctx half
    remains after the context AllGather.

Compute dtype bf16 (validated offline: aw absmax err ~7e-5, av ~9e-3 vs fp32
reference); all accumulations fp32 (PSUM / ACT accumulator / stats math).
"""

from contextlib import ExitStack

import ml_dtypes
import numpy as np

import concourse.bass as bass
import concourse.tile as tile
from concourse import bacc, mybir
from concourse.bass_utils import run_bass_kernel_spmd

F32 = mybir.dt.float32
BF16 = mybir.dt.bfloat16
AF = mybir.ActivationFunctionType
ALU = mybir.AluOpType

N_CORES = 8
S = 16384
H = 4096


def build_graph(n_cores=N_CORES, s_shard=S // N_CORES, h=H, blk=512,
                m_shard=H // N_CORES, act_red=16, dve_red=16, fuse=8):
    """Build the SPMD single-core Bass graph (identical on every core)."""
    nb = s_shard // blk          # score/softmax blocks per core
    ht_tiles = h // 128          # h-tiles (partition tiles)
    pm_cols = h // 128           # columns of the partition-major h_t layout
    jt = (h // n_cores) // 128   # W1 row tiles per core
    k2 = 2 * h // 128            # fc2 contraction tiles
    sub = 4                      # hidden sub-DMAs per block
    assert ht_tiles % sub == 0 and ht_tiles % fuse == 0

    nc = bacc.Bacc("TRN2", target_bir_lowering=False, debug=False,
                   num_devices=n_cores)

    # ---- I/O ----
    hid_t = nc.dram_tensor("hid_t", [h, s_shard], BF16, kind="ExternalInput")
    w1s = nc.dram_tensor("w1s", [h // n_cores, h], BF16, kind="ExternalInput")
    w2t = nc.dram_tensor("w2t", [2 * h, m_shard], BF16, kind="ExternalInput")
    ht_pm = nc.dram_tensor("ht_pm", [128, pm_cols], BF16, kind="ExternalInput")
    ht_loc = nc.dram_tensor("ht_loc", [128, jt], BF16, kind="ExternalInput")
    out_w = nc.dram_tensor("out_w", [s_shard], F32, kind="ExternalOutput")
    out_av = nc.dram_tensor("out_av", [m_shard], F32, kind="ExternalOutput")

    groups = [list(range(n_cores))]

    with tile.TileContext(nc) as tc, ExitStack() as ctx:
        dram = ctx.enter_context(tc.tile_pool(name="dram", bufs=1, space="DRAM"))
        psum = ctx.enter_context(tc.tile_pool(name="psum", bufs=1, space="PSUM"))
        psum2 = ctx.enter_context(tc.tile_pool(name="psum2", bufs=2, space="PSUM"))
        sb = ctx.enter_context(tc.tile_pool(name="sb", bufs=1))
        sb2 = ctx.enter_context(tc.tile_pool(name="sb2", bufs=2))
        hidp = ctx.enter_context(tc.tile_pool(name="hidp", bufs=2))
        w1p = ctx.enter_context(tc.tile_pool(name="w1p", bufs=1))

        ones_bf = sb.tile([1, 128], BF16, name="ones_bf")
        nc.vector.memset(ones_bf[:], 1.0)
        ones_f32 = sb.tile([1, 128], F32, name="ones_f32")
        nc.vector.memset(ones_f32[:], 1.0)

        # Warm-up collective: the first collective of a NEFF absorbs the
        # cross-core launch skew + firmware warm-up (measured 25-50us). Fire a
        # tiny AllGather immediately so the real collectives run at the
        # few-microsecond floor. Reuses the stats buffers (they are
        # overwritten before the real stats AllGather). All small bounce and
        # reload DMAs go on the GPSIMD queue so they never head-of-line-block
        # the big streaming DMAs on the sync queue.
        stats_bounce = dram.tile([2], F32, name="stats_bounce")
        stats_all = dram.tile([2 * n_cores], F32, name="stats_all")
        warm_sb = sb.tile([1, 2], F32, name="warm_sb")
        nc.vector.memset(warm_sb[:], 0.0)
        nc.gpsimd.dma_start(stats_bounce[:].rearrange("(p f) -> p f", p=1),
                            warm_sb[:])
        nc.gpsimd.collective_compute(
            "AllGather", ALU.bypass, replica_groups=groups,
            ins=[stats_bounce.opt()], outs=[stats_all.opt()],
        )

        ht_loc_sb = sb.tile([128, jt], BF16, name="ht_loc_sb")
        nc.sync.dma_start(ht_loc_sb[:], ht_loc.ap())
        ht_pm_sb = sb.tile([128, pm_cols], BF16, name="ht_pm_sb")
        nc.sync.dma_start(ht_pm_sb[:], ht_pm.ap())

        # ---- v = W1.T @ h_t (partial over this core's W1 rows) ----
        # One accumulation group per PSUM bank at a time: each v column gets
        # its own psum tile (2-buf rotation) and is drained by ACT before the
        # bank is re-used.
        w1_sbs = []
        for j in range(jt):
            w1_sb = w1p.tile([128, h], BF16, name=f"w1_sb{j}", tag=f"w1{j}")
            nc.sync.dma_start(w1_sb[:], w1s.ap()[j * 128:(j + 1) * 128, :])
            w1_sbs.append(w1_sb)
        v_sb = sb.tile([128, pm_cols], F32, name="v_sb")
        for i in range(ht_tiles):
            v_ps = psum2.tile([128, 1], F32, name="v_ps", tag="vps")
            for j in range(jt):
                nc.tensor.matmul(
                    v_ps[:],
                    lhsT=w1_sbs[j][:, i * 128:(i + 1) * 128],
                    rhs=ht_loc_sb[:, j:j + 1],
                    start=(j == 0), stop=(j == jt - 1),
                )
            nc.scalar.copy(v_sb[:, i:i + 1], v_ps[:])

        # v partial -> AllGather -> local sum over ranks
        v_bounce = dram.tile([128 * pm_cols], F32, name="v_bounce")
        v_gath = dram.tile([n_cores * 128 * pm_cols], F32, name="v_gath")
        nc.gpsimd.dma_start(v_bounce[:].rearrange("(p t) -> p t", p=128), v_sb[:])
        nc.gpsimd.collective_compute(
            "AllGather", ALU.bypass, replica_groups=groups,
            ins=[v_bounce.opt()], outs=[v_gath.opt()],
        )
        v_all = sb.tile([128, pm_cols * n_cores], F32, name="v_all")
        nc.gpsimd.dma_start(
            v_all[:].rearrange("p (r t) -> p r t", r=n_cores),
            v_gath[:].rearrange("(r p t) -> p r t", p=128, t=pm_cols))
        v_rd = sb.tile([128, pm_cols], F32, name="v_rd")
        nc.vector.tensor_add(v_rd[:], v_all[:, 0:pm_cols],
                             v_all[:, pm_cols:2 * pm_cols])
        for r in range(2, n_cores):
            nc.vector.tensor_add(v_rd[:], v_rd[:],
                                 v_all[:, r * pm_cols:(r + 1) * pm_cols])
        v_pm = sb.tile([128, pm_cols], BF16, name="v_pm")
        nc.scalar.copy(v_pm[:], v_rd[:])

        # ---- main pass over s-blocks: score -> exp -> context partials ----
        e_rows = sb.tile([1, s_shard], BF16, name="e_rows")
        mb_row = sb.tile([1, nb], F32, name="mb_row")
        negmb_row = sb.tile([1, nb], F32, name="negmb_row")
        zb_row = sb.tile([1, nb], F32, name="zb_row")
        ctx_store = sb.tile([128, ht_tiles * nb], F32, name="ctx_store")

        for b in range(nb):
            hid_sb = hidp.tile([128, ht_tiles * blk], BF16, name="hid_sb",
                               tag="hid")
            for g in range(sub):
                tpg = ht_tiles // sub  # h-tiles per sub-DMA
                nc.sync.dma_start(
                    hid_sb[:, g * tpg * blk:(g + 1) * tpg * blk]
                    .rearrange("p (t s) -> p t s", t=tpg),
                    hid_t.ap()[g * tpg * 128:(g + 1) * tpg * 128,
                               b * blk:(b + 1) * blk]
                    .rearrange("(t p) s -> p t s", p=128),
                )

            score_ps = psum2.tile([1, blk], F32, name="score_ps", tag="score")
            for t in range(ht_tiles):
                nc.tensor.matmul(
                    score_ps[:],
                    lhsT=v_pm[:, t:t + 1],
                    rhs=hid_sb[:, t * blk:(t + 1) * blk],
                    start=(t == 0), stop=(t == ht_tiles - 1),
                )

            nc.vector.reduce_max(out=mb_row[:, b:b + 1], in_=score_ps[:],
                                 axis=mybir.AxisListType.X)
            nc.scalar.mul(negmb_row[:, b:b + 1], mb_row[:, b:b + 1], -1.0)
            # e = exp(score - m_b) straight to bf16; Z_b via the ACT
            # accumulator (fp32)
            nc.scalar.activation(
                e_rows[:, b * blk:(b + 1) * blk], score_ps[:], AF.Exp,
                bias=negmb_row[:, b:b + 1], scale=1.0,
                accum_out=zb_row[:, b:b + 1],
            )
            # broadcast e across partitions via PE (bf16)
            e_bc_ps = psum2.tile([128, blk], F32, name="e_bc_ps", tag="ebc")
            nc.tensor.matmul(e_bc_ps[:], lhsT=ones_bf[:],
                             rhs=e_rows[:, b * blk:(b + 1) * blk],
                             start=True, stop=True)
            e_bc = sb2.tile([128, blk], BF16, name="e_bc", tag="ebc_sb")
            nc.scalar.copy(e_bc[:], e_bc_ps[:])

            if b == nb - 1:
                # local stats + stats AllGather — emitted before the trailing
                # context reduces so the collective fires as soon as the last
                # score block is done
                neg_m_core = sb.tile([1, 1], F32, name="neg_m_core")
                nc.vector.tensor_reduce(out=neg_m_core[:], in_=mb_row[:],
                                        axis=mybir.AxisListType.X, op=ALU.max,
                                        negate=True)
                alpha = sb.tile([1, nb], F32, name="alpha")
                nc.scalar.activation(alpha[:], negmb_row[:], AF.Exp,
                                     bias=neg_m_core[:], scale=-1.0)
                scr_nb = sb.tile([1, nb], F32, name="scr_nb")
                z_core = sb.tile([1, 1], F32, name="z_core")
                nc.vector.tensor_mul(scr_nb[:], alpha[:], zb_row[:])
                nc.vector.reduce_sum(out=z_core[:], in_=scr_nb[:],
                                     axis=mybir.AxisListType.X)
                stats_sb = sb.tile([1, 2], F32, name="stats_sb")
                nc.scalar.mul(stats_sb[:, 0:1], neg_m_core[:], -1.0)
                nc.scalar.copy(stats_sb[:, 1:2], z_core[:])
                nc.gpsimd.dma_start(
                    stats_bounce[:].rearrange("(p f) -> p f", p=1),
                    stats_sb[:])
                nc.gpsimd.collective_compute(
                    "AllGather", ALU.bypass, replica_groups=groups,
                    ins=[stats_bounce.opt()], outs=[stats_all.opt()],
                )

            # context partials: ctx_store[:, b*HT + t] = sum_s hid*e
            # DVE does fused multiplies; per-tile free-dim sums split ACT/DVE.
            e_rep = (e_bc[:].rearrange("p (o s) -> p o s", o=1)
                     .broadcast_to((128, fuse, blk)))
            for f in range(ht_tiles // fuse):
                tt_out = sb2.tile([128, fuse * blk], BF16, name="tt_out",
                                  tag="tt", bufs=2)
                nc.vector.tensor_mul(
                    tt_out[:].rearrange("p (t s) -> p t s", t=fuse),
                    hid_sb[:, f * fuse * blk:(f + 1) * fuse * blk]
                    .rearrange("p (t s) -> p t s", t=fuse),
                    e_rep)
                for ti in range(fuse):
                    t = f * fuse + ti
                    col = ctx_store[:, b * ht_tiles + t:b * ht_tiles + t + 1]
                    src = tt_out[:, ti * blk:(ti + 1) * blk]
                    if t < act_red:
                        junk_ps = psum2.tile([128, blk], F32, name="junk_ps",
                                             tag="junk", bufs=1)
                        nc.scalar.activation(junk_ps[:], src, AF.Copy,
                                             accum_out=col)
                    else:
                        nc.vector.reduce_sum(out=col, in_=src,
                                             axis=mybir.AxisListType.X)

        # ---- fc2 h_t half: runs on PE while phase B / collectives proceed
        w2_sb = sb.tile([128, k2 * m_shard], BF16, name="w2_sb")
        w2_sub = 8
        tpg2 = k2 // w2_sub
        for g in range(w2_sub):
            nc.sync.dma_start(
                w2_sb[:, g * tpg2 * m_shard:(g + 1) * tpg2 * m_shard]
                .rearrange("p (t m) -> p t m", t=tpg2),
                w2t.ap()[g * tpg2 * 128:(g + 1) * tpg2 * 128, :]
                .rearrange("(t p) m -> p t m", p=128),
            )
        fc2_ps = psum.tile([1, m_shard], F32, name="fc2_ps")
        for k in range(pm_cols, k2):
            nc.tensor.matmul(
                fc2_ps[:],
                lhsT=ht_pm_sb[:, k - pm_cols:k - pm_cols + 1],
                rhs=w2_sb[:, k * m_shard:(k + 1) * m_shard],
                start=(k == pm_cols), stop=False,
            )

        # ---- global softmax factors from gathered stats ----
        m_all = sb.tile([1, n_cores], F32, name="m_all")
        z_all = sb.tile([1, n_cores], F32, name="z_all")
        strided = stats_all[:].rearrange("(r two) -> two r", two=2)
        nc.gpsimd.dma_start(m_all[:], strided[0:1, :])
        nc.gpsimd.dma_start(z_all[:], strided[1:2, :])

        neg_m_g = sb.tile([1, 1], F32, name="neg_m_g")
        nc.vector.tensor_reduce(out=neg_m_g[:], in_=m_all[:],
                                axis=mybir.AxisListType.X, op=ALU.max,
                                negate=True)
        beta = sb.tile([1, n_cores], F32, name="beta")
        nc.scalar.activation(beta[:], m_all[:], AF.Exp, bias=neg_m_g[:],
                             scale=1.0)
        scr_nc = sb.tile([1, n_cores], F32, name="scr_nc")
        z_g = sb.tile([1, 1], F32, name="z_g")
        nc.vector.tensor_mul(scr_nc[:], beta[:], z_all[:])
        nc.vector.reduce_sum(out=z_g[:], in_=scr_nc[:],
                             axis=mybir.AxisListType.X)
        inv_zg = sb.tile([1, 1], F32, name="inv_zg")
        nc.vector.reciprocal(inv_zg[:], z_g[:])
        # gamma_b = exp(m_b - m_g) / Z_g
        gamma = sb.tile([1, nb], F32, name="gamma")
        nc.scalar.activation(gamma[:], negmb_row[:], AF.Exp, bias=neg_m_g[:],
                             scale=-1.0)
        gamma2 = sb.tile([1, nb], F32, name="gamma2")
        nc.vector.tensor_scalar_mul(gamma2[:], gamma[:], inv_zg[:])

        # ---- attention weights output ----
        w_row = sb.tile([1, s_shard], F32, name="w_row")
        for b in range(nb):
            nc.vector.tensor_scalar_mul(w_row[:, b * blk:(b + 1) * blk],
                                        e_rows[:, b * blk:(b + 1) * blk],
                                        gamma2[:, b:b + 1])
        nc.gpsimd.dma_start(out_w.ap().rearrange("(p f) -> p f", p=1), w_row[:])

        # ---- combine context partials, AllGather + local sum ----
        gam_ps = psum2.tile([128, nb], F32, name="gam_ps", tag="vps")
        nc.tensor.matmul(gam_ps[:], lhsT=ones_f32[:], rhs=gamma2[:],
                         start=True, stop=True)
        gam_sb = sb.tile([128, nb], F32, name="gam_sb")
        nc.scalar.copy(gam_sb[:], gam_ps[:])

        ctx_acc = sb.tile([128, ht_tiles], F32, name="ctx_acc")
        ctx_tmp = sb.tile([128, ht_tiles], F32, name="ctx_tmp")
        nc.vector.tensor_scalar_mul(
            ctx_acc[:], ctx_store[:, 0:ht_tiles], gam_sb[:, 0:1])
        for b in range(1, nb):
            nc.vector.tensor_scalar_mul(
                ctx_tmp[:], ctx_store[:, b * ht_tiles:(b + 1) * ht_tiles],
                gam_sb[:, b:b + 1])
            nc.vector.tensor_add(ctx_acc[:], ctx_acc[:], ctx_tmp[:])

        ctx_bounce = dram.tile([h], F32, name="ctx_bounce")
        ctx_gath = dram.tile([n_cores * h], F32, name="ctx_gath")
        nc.gpsimd.dma_start(ctx_bounce[:].rearrange("(p t) -> p t", p=128),
                          ctx_acc[:])
        nc.gpsimd.collective_compute(
            "AllGather", ALU.bypass, replica_groups=groups,
            ins=[ctx_bounce.opt()], outs=[ctx_gath.opt()],
        )
        ctx_all = sb.tile([128, ht_tiles * n_cores], F32, name="ctx_all")
        nc.gpsimd.dma_start(
            ctx_all[:].rearrange("p (r t) -> p r t", r=n_cores),
            ctx_gath[:].rearrange("(r p t) -> p r t", p=128, t=ht_tiles))
        ctx_rd = sb.tile([128, ht_tiles], F32, name="ctx_rd")
        nc.vector.tensor_add(ctx_rd[:], ctx_all[:, 0:ht_tiles],
                             ctx_all[:, ht_tiles:2 * ht_tiles])
        for r in range(2, n_cores):
            nc.vector.tensor_add(ctx_rd[:], ctx_rd[:],
                                 ctx_all[:, r * ht_tiles:(r + 1) * ht_tiles])

        # ---- fc2 ctx half + tanh ----
        ctx_bf = sb.tile([128, ht_tiles], BF16, name="ctx_bf")
        nc.scalar.copy(ctx_bf[:], ctx_rd[:])
        for k in range(pm_cols):
            nc.tensor.matmul(
                fc2_ps[:],
                lhsT=ctx_bf[:, k:k + 1],
                rhs=w2_sb[:, k * m_shard:(k + 1) * m_shard],
                start=False, stop=(k == pm_cols - 1),
            )
        av_row = sb.tile([1, m_shard], F32, name="av_row")
        nc.scalar.activation(av_row[:], fc2_ps[:], AF.Tanh)
        nc.gpsimd.dma_start(out_av.ap().rearrange("(p f) -> p f", p=1), av_row[:])

    nc.compile()
    return nc


def shard_inputs(hidden_states, W1, W2, n_cores=N_CORES):
    bf = ml_dtypes.bfloat16
    s, h = hidden_states.shape
    s_sh = s // n_cores
    m_sh = h // n_cores
    jt = (h // n_cores) // 128
    ht = np.ascontiguousarray(hidden_states[-1])          # [h] f32
    ht_pm = np.ascontiguousarray(ht.reshape(h // 128, 128).T).astype(bf)
    in_maps = []
    for c in range(n_cores):
        rows = hidden_states[c * s_sh:(c + 1) * s_sh]
        hid_t = np.ascontiguousarray(rows.T).astype(bf)   # [h, s_sh]
        w1s = W1[c * m_sh:(c + 1) * m_sh, :].astype(bf)
        w2t = np.ascontiguousarray(W2[c * m_sh:(c + 1) * m_sh, :].T).astype(bf)
        ht_loc = np.ascontiguousarray(ht_pm[:, c * jt:(c + 1) * jt])
        in_maps.append({
            "hid_t": hid_t, "w1s": w1s, "w2t": w2t,
            "ht_pm": ht_pm, "ht_loc": ht_loc,
        })
    return in_maps


_GRAPH = None
TRACE = False          # set True (e.g. from test.py) to capture an NTFF profile
TMPDIR = None          # optional trace output dir
LAST_RESULTS = None    # BassKernelResults of the most recent run


def kernel(hidden_states, W1, W2):
    global _GRAPH, LAST_RESULTS
    hidden_states = np.asarray(hidden_states, dtype=np.float32)
    W1 = np.asarray(W1, dtype=np.float32)
    W2 = np.asarray(W2, dtype=np.float32)
    if _GRAPH is None:
        _GRAPH = build_graph()
    in_maps = shard_inputs(hidden_states, W1, W2)
    res = run_bass_kernel_spmd(_GRAPH, in_maps, core_ids=list(range(N_CORES)),
                               trace=TRACE, tmpdir=TMPDIR)
    LAST_RESULTS = res
    outs = res.results
    aw = np.concatenate([outs[c]["out_w"] for c in range(N_CORES)])
    av = np.concatenate([outs[c]["out_av"] for c in range(N_CORES)])
    return av.astype(np.float32), aw.astype(np.float32)


# revision 26
# speedup vs baseline: 1.4403x; 1.0734x over previous
"""Distributed Trainium2 Bass kernel for nn_Attention_72791105732731.

Reference computation (S=16384, H=4096):
    score_ = hidden @ W1.T            # [S,H]
    h_t    = hidden[-1]
    score  = score_ @ h_t             # [S]
    aw     = softmax(score)
    ctx    = hidden.T @ aw            # [H]
    av     = tanh(W2 @ concat(ctx, h_t))
    return (av, aw)

Key algebraic identity: score = hidden @ (W1.T @ h_t) — reassociation turns the
550-GFLOP fc1 matmul into two matvecs, making the problem memory-bound.

Distribution over 8 cores:
  - hidden sharded over S (2048 rows/core), host-pre-transposed to [H, 2048]
    so the score contraction (over H) sits on the partition axis for TensorE.
  - W1 sharded over rows (512/core): partial v = W1_shard.T @ h_t_shard,
    AllGather + local sum (cheaper than AllReduce at this size).
  - softmax via block-local (max, sumexp) stats + AllGather of per-core stats.
  - context partials: DVE multiplies (bf16 2x mode, fused 8 tiles/op), free-dim
    sums split between the Scalar engine (activation accum_out) and GPSIMD so
    no single engine paces the pass; AllGather + local sum for the context.
  - W2 sharded over output rows (512/core), host-pre-transposed; the h_t half
    of fc2 is accumulated into PSUM during the main pass, only the ctx half
    remains after the context AllGather.

Compute dtype bf16 (validated offline: aw absmax err ~7e-5, av ~9e-3 vs fp32
reference); all accumulations fp32 (PSUM / ACT accumulator / stats math).
"""

from contextlib import ExitStack

import ml_dtypes
import numpy as np

import concourse.bass as bass
import concourse.tile as tile
from concourse import bacc, mybir
from concourse.bass_utils import run_bass_kernel_spmd

F32 = mybir.dt.float32
BF16 = mybir.dt.bfloat16
AF = mybir.ActivationFunctionType
ALU = mybir.AluOpType

N_CORES = 8
S = 16384
H = 4096


def build_graph(n_cores=N_CORES, s_shard=S // N_CORES, h=H, blk=512,
                m_shard=H // N_CORES, act_red=16, dve_red=16, fuse=8):
    """Build the SPMD single-core Bass graph (identical on every core)."""
    nb = s_shard // blk          # score/softmax blocks per core
    ht_tiles = h // 128          # h-tiles (partition tiles)
    pm_cols = h // 128           # columns of the partition-major h_t layout
    jt = (h // n_cores) // 128   # W1 row tiles per core
    k2 = 2 * h // 128            # fc2 contraction tiles
    sub = 4                      # hidden sub-DMAs per block
    assert ht_tiles % sub == 0 and ht_tiles % fuse == 0

    nc = bacc.Bacc("TRN2", target_bir_lowering=False, debug=False,
                   num_devices=n_cores)

    # ---- I/O ----
    hid_t = nc.dram_tensor("hid_t", [h, s_shard], BF16, kind="ExternalInput")
    w1s = nc.dram_tensor("w1s", [h // n_cores, h], BF16, kind="ExternalInput")
    w2t = nc.dram_tensor("w2t", [2 * h, m_shard], BF16, kind="ExternalInput")
    ht_pm = nc.dram_tensor("ht_pm", [128, pm_cols], BF16, kind="ExternalInput")
    ht_loc = nc.dram_tensor("ht_loc", [128, jt], BF16, kind="ExternalInput")
    out_w = nc.dram_tensor("out_w", [s_shard], F32, kind="ExternalOutput")
    out_av = nc.dram_tensor("out_av", [m_shard], F32, kind="ExternalOutput")

    groups = [list(range(n_cores))]

    with tile.TileContext(nc) as tc, ExitStack() as ctx:
        dram = ctx.enter_context(tc.tile_pool(name="dram", bufs=1, space="DRAM"))
        psum = ctx.enter_context(tc.tile_pool(name="psum", bufs=1, space="PSUM"))
        psum2 = ctx.enter_context(tc.tile_pool(name="psum2", bufs=2, space="PSUM"))
        sb = ctx.enter_context(tc.tile_pool(name="sb", bufs=1))
        sb2 = ctx.enter_context(tc.tile_pool(name="sb2", bufs=2))
        hidp = ctx.enter_context(tc.tile_pool(name="hidp", bufs=2))
        w1p = ctx.enter_context(tc.tile_pool(name="w1p", bufs=1))

        ones_bf = sb.tile([1, 128], BF16, name="ones_bf")
        nc.vector.memset(ones_bf[:], 1.0)
        ones_f32 = sb.tile([1, 128], F32, name="ones_f32")
        nc.vector.memset(ones_f32[:], 1.0)

        ht_loc_sb = sb.tile([128, jt], BF16, name="ht_loc_sb")
        nc.sync.dma_start(ht_loc_sb[:], ht_loc.ap())
        ht_pm_sb = sb.tile([128, pm_cols], BF16, name="ht_pm_sb")
        nc.sync.dma_start(ht_pm_sb[:], ht_pm.ap())

        # ---- v = W1.T @ h_t (partial over this core's W1 rows) ----
        # One accumulation group per PSUM bank at a time: each v column gets
        # its own psum tile (2-buf rotation) and is drained by ACT before the
        # bank is re-used.
        w1_sbs = []
        for j in range(jt):
            w1_sb = w1p.tile([128, h], BF16, name=f"w1_sb{j}", tag=f"w1{j}")
            nc.sync.dma_start(w1_sb[:], w1s.ap()[j * 128:(j + 1) * 128, :])
            w1_sbs.append(w1_sb)
        v_sb = sb.tile([128, pm_cols], F32, name="v_sb")
        for i in range(ht_tiles):
            v_ps = psum2.tile([128, 1], F32, name="v_ps", tag="vps")
            for j in range(jt):
                nc.tensor.matmul(
                    v_ps[:],
                    lhsT=w1_sbs[j][:, i * 128:(i + 1) * 128],
                    rhs=ht_loc_sb[:, j:j + 1],
                    start=(j == 0), stop=(j == jt - 1),
                )
            nc.scalar.copy(v_sb[:, i:i + 1], v_ps[:])

        # v partial -> AllGather -> local sum over ranks
        v_bounce = dram.tile([128 * pm_cols], F32, name="v_bounce")
        v_gath = dram.tile([n_cores * 128 * pm_cols], F32, name="v_gath")
        nc.gpsimd.dma_start(v_bounce[:].rearrange("(p t) -> p t", p=128), v_sb[:])
        nc.gpsimd.collective_compute(
            "AllGather", ALU.bypass, replica_groups=groups,
            ins=[v_bounce.opt()], outs=[v_gath.opt()],
        )
        v_all = sb.tile([128, pm_cols * n_cores], F32, name="v_all")
        nc.gpsimd.dma_start(
            v_all[:].rearrange("p (r t) -> p r t", r=n_cores),
            v_gath[:].rearrange("(r p t) -> p r t", p=128, t=pm_cols))
        v_rd = sb.tile([128, pm_cols], F32, name="v_rd")
        nc.vector.tensor_add(v_rd[:], v_all[:, 0:pm_cols],
                             v_all[:, pm_cols:2 * pm_cols])
        for r in range(2, n_cores):
            nc.vector.tensor_add(v_rd[:], v_rd[:],
                                 v_all[:, r * pm_cols:(r + 1) * pm_cols])
        v_pm = sb.tile([128, pm_cols], BF16, name="v_pm")
        nc.scalar.copy(v_pm[:], v_rd[:])

        # ---- main pass over s-blocks: score -> exp -> context partials ----
        e_rows = sb.tile([1, s_shard], BF16, name="e_rows")
        mb_row = sb.tile([1, nb], F32, name="mb_row")
        negmb_row = sb.tile([1, nb], F32, name="negmb_row")
        zb_row = sb.tile([1, nb], F32, name="zb_row")
        ctx_store = sb.tile([128, ht_tiles * nb], F32, name="ctx_store")

        for b in range(nb):
            hid_sb = hidp.tile([128, ht_tiles * blk], BF16, name="hid_sb",
                               tag="hid")
            for g in range(sub):
                tpg = ht_tiles // sub  # h-tiles per sub-DMA
                nc.sync.dma_start(
                    hid_sb[:, g * tpg * blk:(g + 1) * tpg * blk]
                    .rearrange("p (t s) -> p t s", t=tpg),
                    hid_t.ap()[g * tpg * 128:(g + 1) * tpg * 128,
                               b * blk:(b + 1) * blk]
                    .rearrange("(t p) s -> p t s", p=128),
                )

            score_ps = psum2.tile([1, blk], F32, name="score_ps", tag="score")
            for t in range(ht_tiles):
                nc.tensor.matmul(
                    score_ps[:],
                    lhsT=v_pm[:, t:t + 1],
                    rhs=hid_sb[:, t * blk:(t + 1) * blk],
                    start=(t == 0), stop=(t == ht_tiles - 1),
                )

            nc.vector.reduce_max(out=mb_row[:, b:b + 1], in_=score_ps[:],
                                 axis=mybir.AxisListType.X)
            nc.scalar.mul(negmb_row[:, b:b + 1], mb_row[:, b:b + 1], -1.0)
            # e = exp(score - m_b) straight to bf16; Z_b via the ACT
            # accumulator (fp32)
            nc.scalar.activation(
                e_rows[:, b * blk:(b + 1) * blk], score_ps[:], AF.Exp,
                bias=negmb_row[:, b:b + 1], scale=1.0,
                accum_out=zb_row[:, b:b + 1],
            )
            # broadcast e across partitions via PE (bf16)
            e_bc_ps = psum2.tile([128, blk], F32, name="e_bc_ps", tag="ebc")
            nc.tensor.matmul(e_bc_ps[:], lhsT=ones_bf[:],
                             rhs=e_rows[:, b * blk:(b + 1) * blk],
                             start=True, stop=True)
            e_bc = sb2.tile([128, blk], BF16, name="e_bc", tag="ebc_sb")
            nc.scalar.copy(e_bc[:], e_bc_ps[:])

            if b == nb - 1:
                # local stats (block-combine factors) — emitted before the
                # trailing context reduces so they are ready early
                neg_m_core = sb.tile([1, 1], F32, name="neg_m_core")
                nc.vector.tensor_reduce(out=neg_m_core[:], in_=mb_row[:],
                                        axis=mybir.AxisListType.X, op=ALU.max,
                                        negate=True)
                alpha = sb.tile([1, nb], F32, name="alpha")
                nc.scalar.activation(alpha[:], negmb_row[:], AF.Exp,
                                     bias=neg_m_core[:], scale=-1.0)
                scr_nb = sb.tile([1, nb], F32, name="scr_nb")
                z_core = sb.tile([1, 1], F32, name="z_core")
                nc.vector.tensor_mul(scr_nb[:], alpha[:], zb_row[:])
                nc.vector.reduce_sum(out=z_core[:], in_=scr_nb[:],
                                     axis=mybir.AxisListType.X)
                stats8 = sb.tile([1, 8], F32, name="stats8")
                nc.vector.memset(stats8[:], 0.0)
                nc.scalar.mul(stats8[:, 0:1], neg_m_core[:], -1.0)
                nc.scalar.copy(stats8[:, 1:2], z_core[:])
                # combine context partials across blocks with alpha_b =
                # exp(m_b - m_core), ready for the merged AllGather
                alpha_ps = psum2.tile([128, nb], F32, name="alpha_ps",
                                      tag="vps")
                nc.tensor.matmul(alpha_ps[:], lhsT=ones_f32[:], rhs=alpha[:],
                                 start=True, stop=True)
                alpha_sb = sb.tile([128, nb], F32, name="alpha_sb")
                nc.scalar.copy(alpha_sb[:], alpha_ps[:])

            # context partials: ctx_store[:, b*HT + t] = sum_s hid*e
            # DVE does fused multiplies; per-tile free-dim sums split ACT/DVE.
            e_rep = (e_bc[:].rearrange("p (o s) -> p o s", o=1)
                     .broadcast_to((128, fuse, blk)))
            for f in range(ht_tiles // fuse):
                tt_out = sb2.tile([128, fuse * blk], BF16, name="tt_out",
                                  tag="tt", bufs=2)
                nc.vector.tensor_mul(
                    tt_out[:].rearrange("p (t s) -> p t s", t=fuse),
                    hid_sb[:, f * fuse * blk:(f + 1) * fuse * blk]
                    .rearrange("p (t s) -> p t s", t=fuse),
                    e_rep)
                for ti in range(fuse):
                    t = f * fuse + ti
                    col = ctx_store[:, b * ht_tiles + t:b * ht_tiles + t + 1]
                    src = tt_out[:, ti * blk:(ti + 1) * blk]
                    if t < act_red:
                        junk_ps = psum2.tile([128, blk], F32, name="junk_ps",
                                             tag="junk", bufs=1)
                        nc.scalar.activation(junk_ps[:], src, AF.Copy,
                                             accum_out=col)
                    else:
                        nc.vector.reduce_sum(out=col, in_=src,
                                             axis=mybir.AxisListType.X)

        # ---- fc2 h_t half: runs on PE while collectives proceed ----
        w2_sb = sb.tile([128, k2 * m_shard], BF16, name="w2_sb")
        w2_sub = 8
        tpg2 = k2 // w2_sub
        for g in range(w2_sub):
            nc.sync.dma_start(
                w2_sb[:, g * tpg2 * m_shard:(g + 1) * tpg2 * m_shard]
                .rearrange("p (t m) -> p t m", t=tpg2),
                w2t.ap()[g * tpg2 * 128:(g + 1) * tpg2 * 128, :]
                .rearrange("(t p) m -> p t m", p=128),
            )
        fc2_ps = psum.tile([1, m_shard], F32, name="fc2_ps")
        for k in range(pm_cols, k2):
            nc.tensor.matmul(
                fc2_ps[:],
                lhsT=ht_pm_sb[:, k - pm_cols:k - pm_cols + 1],
                rhs=w2_sb[:, k * m_shard:(k + 1) * m_shard],
                start=(k == pm_cols), stop=False,
            )

        # ---- block-combine context, one merged AllGather of [ctx | stats] --
        ctx_acc = sb.tile([128, ht_tiles], F32, name="ctx_acc")
        ctx_tmp = sb.tile([128, ht_tiles], F32, name="ctx_tmp")
        nc.vector.tensor_scalar_mul(
            ctx_acc[:], ctx_store[:, 0:ht_tiles], alpha_sb[:, 0:1])
        for b in range(1, nb):
            nc.vector.tensor_scalar_mul(
                ctx_tmp[:], ctx_store[:, b * ht_tiles:(b + 1) * ht_tiles],
                alpha_sb[:, b:b + 1])
            nc.vector.tensor_add(ctx_acc[:], ctx_acc[:], ctx_tmp[:])

        chunk = h + 8  # 32B-aligned per-rank payload: ctx (pm) + stats + pad
        mg_bounce = dram.tile([chunk], F32, name="mg_bounce")
        mg_all = dram.tile([n_cores * chunk], F32, name="mg_all")
        nc.gpsimd.dma_start(
            mg_bounce[:][0:h].rearrange("(p t) -> p t", p=128), ctx_acc[:])
        nc.gpsimd.dma_start(
            mg_bounce[:][h:h + 8].rearrange("(p f) -> p f", p=1), stats8[:])
        nc.gpsimd.collective_compute(
            "AllGather", ALU.bypass, replica_groups=groups,
            ins=[mg_bounce.opt()], outs=[mg_all.opt()],
        )
        per_rank = mg_all[:].rearrange("(r x) -> r x", r=n_cores)
        ctx_all = sb.tile([128, ht_tiles * n_cores], F32, name="ctx_all")
        nc.gpsimd.dma_start(
            ctx_all[:].rearrange("p (r t) -> p r t", r=n_cores),
            per_rank[:, 0:h].rearrange("r (p t) -> p r t", p=128))
        m_all = sb.tile([1, n_cores], F32, name="m_all")
        z_all = sb.tile([1, n_cores], F32, name="z_all")
        nc.gpsimd.dma_start(
            m_all[:], per_rank[:, h:h + 1].rearrange("r one -> one r"))
        nc.gpsimd.dma_start(
            z_all[:], per_rank[:, h + 1:h + 2].rearrange("r one -> one r"))

        # ---- global softmax factors ----
        neg_m_g = sb.tile([1, 1], F32, name="neg_m_g")
        nc.vector.tensor_reduce(out=neg_m_g[:], in_=m_all[:],
                                axis=mybir.AxisListType.X, op=ALU.max,
                                negate=True)
        beta = sb.tile([1, n_cores], F32, name="beta")
        nc.scalar.activation(beta[:], m_all[:], AF.Exp, bias=neg_m_g[:],
                             scale=1.0)
        scr_nc = sb.tile([1, n_cores], F32, name="scr_nc")
        z_g = sb.tile([1, 1], F32, name="z_g")
        nc.vector.tensor_mul(scr_nc[:], beta[:], z_all[:])
        nc.vector.reduce_sum(out=z_g[:], in_=scr_nc[:],
                             axis=mybir.AxisListType.X)
        inv_zg = sb.tile([1, 1], F32, name="inv_zg")
        nc.vector.reciprocal(inv_zg[:], z_g[:])
        # per-rank gamma_r = exp(m_r - m_g) / Z_g, broadcast to partitions
        gamma_r = sb.tile([1, n_cores], F32, name="gamma_r")
        nc.vector.tensor_scalar_mul(gamma_r[:], beta[:], inv_zg[:])
        gamr_ps = psum2.tile([128, n_cores], F32, name="gamr_ps", tag="vps")
        nc.tensor.matmul(gamr_ps[:], lhsT=ones_f32[:], rhs=gamma_r[:],
                         start=True, stop=True)
        gamr_sb = sb.tile([128, n_cores], F32, name="gamr_sb")
        nc.scalar.copy(gamr_sb[:], gamr_ps[:])

        # ---- attention weights output: w = e * exp(m_b - m_g)/Z_g ----
        gamma_b = sb.tile([1, nb], F32, name="gamma_b")
        nc.scalar.activation(gamma_b[:], negmb_row[:], AF.Exp, bias=neg_m_g[:],
                             scale=-1.0)
        gamma_b2 = sb.tile([1, nb], F32, name="gamma_b2")
        nc.vector.tensor_scalar_mul(gamma_b2[:], gamma_b[:], inv_zg[:])
        w_row = sb.tile([1, s_shard], F32, name="w_row")
        for b in range(nb):
            nc.vector.tensor_scalar_mul(w_row[:, b * blk:(b + 1) * blk],
                                        e_rows[:, b * blk:(b + 1) * blk],
                                        gamma_b2[:, b:b + 1])
        nc.gpsimd.dma_start(out_w.ap().rearrange("(p f) -> p f", p=1), w_row[:])

        # ---- rank-combine context ----
        ctx_rd = sb.tile([128, ht_tiles], F32, name="ctx_rd")
        ctx_rt = sb.tile([128, ht_tiles], F32, name="ctx_rt")
        nc.vector.tensor_scalar_mul(ctx_rd[:], ctx_all[:, 0:ht_tiles],
                                    gamr_sb[:, 0:1])
        for r in range(1, n_cores):
            nc.vector.tensor_scalar_mul(
                ctx_rt[:], ctx_all[:, r * ht_tiles:(r + 1) * ht_tiles],
                gamr_sb[:, r:r + 1])
            nc.vector.tensor_add(ctx_rd[:], ctx_rd[:], ctx_rt[:])

        # ---- fc2 ctx half + tanh ----
        ctx_bf = sb.tile([128, ht_tiles], BF16, name="ctx_bf")
        nc.scalar.copy(ctx_bf[:], ctx_rd[:])
        for k in range(pm_cols):
            nc.tensor.matmul(
                fc2_ps[:],
                lhsT=ctx_bf[:, k:k + 1],
                rhs=w2_sb[:, k * m_shard:(k + 1) * m_shard],
                start=False, stop=(k == pm_cols - 1),
            )
        av_row = sb.tile([1, m_shard], F32, name="av_row")
        nc.scalar.activation(av_row[:], fc2_ps[:], AF.Tanh)
        nc.gpsimd.dma_start(out_av.ap().rearrange("(p f) -> p f", p=1), av_row[:])

    nc.compile()
    return nc


def shard_inputs(hidden_states, W1, W2, n_cores=N_CORES):
    bf = ml_dtypes.bfloat16
    s, h = hidden_states.shape
    s_sh = s // n_cores
    m_sh = h // n_cores
    jt = (h // n_cores) // 128
    ht = np.ascontiguousarray(hidden_states[-1])          # [h] f32
    ht_pm = np.ascontiguousarray(ht.reshape(h // 128, 128).T).astype(bf)
    in_maps = []
    for c in range(n_cores):
        rows = hidden_states[c * s_sh:(c + 1) * s_sh]
        hid_t = np.ascontiguousarray(rows.T).astype(bf)   # [h, s_sh]
        w1s = W1[c * m_sh:(c + 1) * m_sh, :].astype(bf)
        w2t = np.ascontiguousarray(W2[c * m_sh:(c + 1) * m_sh, :].T).astype(bf)
        ht_loc = np.ascontiguousarray(ht_pm[:, c * jt:(c + 1) * jt])
        in_maps.append({
            "hid_t": hid_t, "w1s": w1s, "w2t": w2t,
            "ht_pm": ht_pm, "ht_loc": ht_loc,
        })
    return in_maps


_GRAPH = None
TRACE = False          # set True (e.g. from test.py) to capture an NTFF profile
TMPDIR = None          # optional trace output dir
LAST_RESULTS = None    # BassKernelResults of the most recent run


def kernel(hidden_states, W1, W2):
    global _GRAPH, LAST_RESULTS
    hidden_states = np.asarray(hidden_states, dtype=np.float32)
    W1 = np.asarray(W1, dtype=np.float32)
    W2 = np.asarray(W2, dtype=np.float32)
    if _GRAPH is None:
        _GRAPH = build_graph()
    in_maps = shard_inputs(hidden_states, W1, W2)
    res = run_bass_kernel_spmd(_GRAPH, in_maps, core_ids=list(range(N_CORES)),
                               trace=TRACE, tmpdir=TMPDIR)
    LAST_RESULTS = res
    outs = res.results
    aw = np.concatenate([outs[c]["out_w"] for c in range(N_CORES)])
    av = np.concatenate([outs[c]["out_av"] for c in range(N_CORES)])
    return av.astype(np.float32), aw.astype(np.float32)
